# revision 49
# baseline (speedup 1.0000x reference)
"""Trainium2 Bass kernel for 2-layer GAT (nn_GAT_43765716746408).

Self-contained: hardcodes the problem geometry (50000 nodes, 800000 edges,
D=128, H=2 heads, F=128, 2 layers) and distributes across 8 NeuronCores by
dst-node partition.

Strategy per layer (SPMD across 8 cores, identical program, per-core data):
  - Replicated feature matmul (P1): every core computes feat = h @ [W|wl|wr]
    for ALL nodes (wl/wr fold the attention vectors al/ar into the matmul so
    el = feat@al, er = feat@ar come out as 4 extra columns), writing a packed
    row table T[n] = [feat fp16 (512B) | el f32 (8B) | pad] (768B rows),
    split into two DRAM tables (lo/hi node halves) so edge gathers on the lo
    half can start while the hi half is still being written.  er for the
    core's own nodes is kept in SBUF (er_own).
  - Edge phase (P2): edges are bucketed by (dst-tile-of-128, src-half) on the
    host (padded with dummy edges to uniform bucket sizes shared by all
    cores), and buckets are grouped into batches of a few dst tiles so each
    dma_gather instruction covers thousands of edges (SWDGE descriptor
    generation on GpSimd has a large per-instruction cost).  Per dst tile:
    one-hot S[e, d] = (dst[e] == d) via iota + is_equal, transposed one-hot
    S_T[d, e] via a host-replicated dst table + partition iota, er per edge
    via a small matmul (S_T^T @ er_own), w = exp(lrelu(el + er)) on the
    Scalar engine written directly into msg columns 256:258, msg = w * feat
    on Vector, and out[d] = sum_e S[e,d] * msg[e] via one 128x258 matmul per
    128-edge group accumulated in PSUM.  Finalize divides by the accumulated
    denominators (msg cols 256:258 aggregate to per-dst sums of w) and adds
    bias.
  - Between layers: h = mean over heads, transposed on-chip to (feat, node)
    layout and AllGather'd so every core has the full h for layer 2's
    replicated matmul.
"""

import sys

sys.path.insert(0, "/opt/trn_rl_repo")

import numpy as np

import concourse.bass as bass
import concourse.tile as tile
from concourse import bacc, mybir
from concourse.bass_utils import run_bass_kernel_spmd
from concourse.masks import make_identity

F32 = mybir.dt.float32
F16 = mybir.dt.float16
I16 = mybir.dt.int16

N_CORES = 8
D = 128          # model dim
H = 2            # heads
HF = 256         # H * F
ROW16 = 384      # fp16 elements per table row (512B feat + 8B el + pad = 768B)
NEG_SLOPE = 0.2
B_TILES = 2      # dst tiles per gather batch
P1B = 4          # node tiles per P1 load/store batch


class Cfg:
    def __init__(self, n_nodes, n_edges, n_layers=2):
        assert n_nodes % N_CORES == 0
        self.n = n_nodes
        self.e = n_edges
        self.layers = n_layers
        self.nloc = n_nodes // N_CORES
        self.t = -(-self.nloc // 128)          # dst tiles per core
        self.nloc_pad = self.t * 128
        self.w_last = self.nloc - 128 * (self.t - 1)
        self.split = n_nodes // 2              # lo/hi table split (int16 range)
        assert self.split < 32768 and (n_nodes - self.split) < 32768
        assert self.nloc_pad < 32768


FULL = Cfg(50000, 800000)


# ----------------------------------------------------------------------------
# Host-side edge preprocessing
# ----------------------------------------------------------------------------

def prep_edges(cfg, src, dst):
    """Bucket edges per core by (dst_tile, src_half); pad to shared sizes.

    Slot order is batch-major: for each batch of B_TILES dst tiles, first all
    lo buckets of the batch's tiles, then all hi buckets, so one dma_gather
    per (batch, half) covers a contiguous slot range.
    """
    C, T = N_CORES, cfg.t
    counts = np.zeros((C, T, 2), dtype=np.int64)
    per_core = []
    core_of = dst // cfg.nloc
    for c in range(C):
        sel = core_of == c
        es, ed = src[sel].astype(np.int64), dst[sel].astype(np.int64)
        dloc = ed - c * cfg.nloc
        t = dloc // 128
        half = (es >= cfg.split).astype(np.int64)
        # sort by (tile, half, src) for gather locality
        order = np.lexsort((es, half, t))
        es, dloc, t, half = es[order], dloc[order], t[order], half[order]
        np.add.at(counts[c], (t, half), 1)
        per_core.append((es, dloc, t, half))

    gmax_th = counts.max(axis=0)                       # (T, 2)
    G = np.maximum(1, -(-gmax_th // 128))              # groups per (t, half)

    batches = [(t0, min(B_TILES, T - t0)) for t0 in range(0, T, B_TILES)]
    base = np.zeros((T, 2), dtype=np.int64)            # group offset per bucket
    acc = 0
    binfo = []                                         # (g_lo0, g_hi0, g_end)
    for (t0, nt) in batches:
        g_lo0 = acc
        for t in range(t0, t0 + nt):
            base[t, 0] = acc
            acc += G[t, 0]
        g_hi0 = acc
        for t in range(t0, t0 + nt):
            base[t, 1] = acc
            acc += G[t, 1]
        binfo.append((g_lo0, g_hi0, acc))
    gtot = acc
    nslot = gtot * 128

    src_idx = np.zeros((C, nslot), dtype=np.int16)
    er_idx = np.zeros((C, nslot), dtype=np.int16)
    dst_reb = np.full((C, nslot), -1.0, dtype=np.float16)
    for c in range(C):
        es, dloc, t, half = per_core[c]
        # edges are lexsorted by (t, half, src) so buckets are contiguous:
        # position within bucket = arange - bucket start
        bucket_id = t * 2 + half
        n = len(es)
        starts = np.searchsorted(bucket_id, np.arange(T * 2), side="left")
        pos_in_bucket = np.arange(n) - starts[bucket_id]
        slot = base[t, half] * 128 + pos_in_bucket
        src_idx[c, slot] = (es - np.where(half == 1, cfg.split, 0)).astype(np.int16)
        er_idx[c, slot] = dloc.astype(np.int16)
        dst_reb[c, slot] = (dloc - t * 128).astype(np.float32)

    # wrapped int16 index layout: element s -> [s % 16, s // 16], replicated
    # to 128 partitions (the 8 gpsimd cores each read their 16-partition copy)
    def wrap16(a):
        w = a.reshape(-1, 16).T.copy()                 # (16, nslot/16)
        return np.tile(w, (8, 1))                      # (128, nslot/16)

    src_w = np.stack([wrap16(src_idx[c]) for c in range(C)])
    er_w = np.stack([wrap16(er_idx[c]) for c in range(C)])
    # host-precomputed one-hot aggregation matrix (layer-invariant, streamed
    # from DRAM), stored partition-contiguous: s_tab[e, g*128 + d] =
    # (dst[g*128+e] == d) so each device load is 128 large contiguous
    # descriptors, not 128*cnt small strided ones.
    dgrid = np.arange(D, dtype=np.float16)
    s_tab = np.stack(
        [
            (dst_reb[c].reshape(gtot, 128)[:, :, None] == dgrid[None, None, :])
            .astype(np.float16).transpose(1, 0, 2).reshape(D, nslot)
            for c in range(C)
        ]
    )                                                  # (C, 128, nslot)

    geom = {
        "G": G,
        "base": base,
        "gtot": gtot,
        "batches": batches,
        "binfo": binfo,
        "gt_max": int((G[:, 0] + G[:, 1]).max()),
        "gb_max": int(max(e - s for (s, _h, e) in binfo)),
    }
    return geom, src_w, er_w, s_tab


def prep_weights(cfg, Ws, als, ars, bs):
    """Combined matmul weights Wc = [W | wl | wr] and packed bias rows.

    W[l] is (D, H*F) with head-major columns; wl[k,h] = sum_f W[k,h,f]*al[h,f]
    folds the attention dot products into the same matmul.
    """
    L = cfg.layers
    wc = np.zeros((L, D, HF + 4), dtype=np.float16)
    bp = np.zeros((L, 1, 384), dtype=np.float16)
    for l in range(L):
        W = np.asarray(Ws[l], dtype=np.float32)            # (D, H*F)
        Wh = W.reshape(D, H, D)                            # (D, H, F)
        wl = np.einsum("khf,hf->kh", Wh, np.asarray(als[l], np.float32))
        wr = np.einsum("khf,hf->kh", Wh, np.asarray(ars[l], np.float32))
        wc[l, :, :HF] = W.astype(np.float16)
        wc[l, :, HF : HF + 2] = wl.astype(np.float16)
        wc[l, :, HF + 2 : HF + 4] = wr.astype(np.float16)
        b = np.asarray(bs[l], np.float32)                  # (H, F)
        bp[l, 0, 0:128] = b[0].astype(np.float16)
        bp[l, 0, 128:256] = b[1].astype(np.float16)
        bp[l, 0, 256:384] = (0.5 * (b[0] + b[1])).astype(np.float16)
    return wc, bp


# ----------------------------------------------------------------------------
# Device kernel
# ----------------------------------------------------------------------------

def build(cfg, geom):
    C, T, L = N_CORES, cfg.t, cfg.layers
    G, base = geom["G"], geom["base"]
    gtot, batches, binfo = geom["gtot"], geom["batches"], geom["binfo"]
    GT_MAX, GB_MAX = geom["gt_max"], geom["gb_max"]
    nslot = gtot * 128
    NLO, NHI = cfg.split, cfg.n - cfg.split

    nc = bacc.Bacc("TRN2", target_bir_lowering=False, debug=False,
                   enable_asserts=False, num_devices=C, num_swdge_queues=2,
                   dynamic_dma_scratch_size=32768)

    # I/O
    xTb = nc.dram_tensor("xTb", [C, D, cfg.nloc], F16, kind="ExternalInput")
    wc_d = nc.dram_tensor("wc", [L, D, HF + 4], F16, kind="ExternalInput")
    bp_d = nc.dram_tensor("bp", [L, 1, 384], F16, kind="ExternalInput")
    src_d = nc.dram_tensor("srcw", [D, nslot // 16], I16, kind="ExternalInput")
    er_d = nc.dram_tensor("erw", [D, nslot // 16], I16, kind="ExternalInput")
    s_d = nc.dram_tensor("s_tab", [D, nslot], F16, kind="ExternalInput")
    oh_d = nc.dram_tensor("onehot", [D, 8], F32, kind="ExternalInput")
    # raw layer-2 accumulators: [num_h0 | num_h1 | den_h0 | den_h1];
    # normalization + bias happen on the host
    out_d = nc.dram_tensor("out", [cfg.nloc_pad, 258], F32, kind="ExternalOutput")

    # internal DRAM
    tb_lo = nc.dram_tensor("tb_lo", [NLO, ROW16], F16)
    tb_hi = nc.dram_tensor("tb_hi", [NHI, ROW16], F16)
    tsmall = nc.dram_tensor("tsmall", [cfg.nloc_pad, 64], F32)
    hT_own = nc.dram_tensor("hT_own", [D, cfg.nloc], F16)
    hT_all = nc.dram_tensor("hT_all", [C, D, cfg.nloc], F16, addr_space="Shared")

    assert NLO == 4 * cfg.nloc, "lo half must be cores 0..3"

    with tile.TileContext(nc) as tc:
        with (
            tc.tile_pool(name="const", bufs=1) as cpool,
            tc.tile_pool(name="work", bufs=2) as pool,
            tc.tile_pool(name="ps_a", bufs=3, space="PSUM") as ppa,
            tc.tile_pool(name="ps_b", bufs=2, space="PSUM") as ppb,
        ):
            # ---- constants ----
            src_sb = cpool.tile([D, nslot // 16], I16, tag="src_sb")
            nc.sync.dma_start(out=src_sb[:], in_=src_d[:])
            er_sb = cpool.tile([D, nslot // 16], I16, tag="er_sb")
            nc.sync.dma_start(out=er_sb[:], in_=er_d[:])
            oh_sb = cpool.tile([D, 8], F32, tag="oh_sb")
            nc.sync.dma_start(out=oh_sb[:], in_=oh_d[:])
            wc_sb = cpool.tile([D, L * (HF + 4)], F16, tag="wc_sb")
            bp_sb = cpool.tile([1, L * 384], F16, tag="bp_sb")
            for l in range(L):
                nc.sync.dma_start(
                    out=wc_sb[:, l * (HF + 4) : (l + 1) * (HF + 4)], in_=wc_d[l]
                )
                nc.sync.dma_start(
                    out=bp_sb[:, l * 384 : (l + 1) * 384], in_=bp_d[l]
                )

            ones_row = cpool.tile([1, D], F16, tag="ones_row")
            nc.vector.memset(ones_row[:], 1.0)
            ident = cpool.tile([D, D], F16, tag="ident")
            make_identity(nc, ident[:])

            er_stage = cpool.tile([D, T, 16], F32, tag="er_stage")
            er_own = cpool.tile([D, T, 2], F32, tag="er_own")
            hT_stage = cpool.tile([D, cfg.nloc_pad], F16, tag="hT_stage")
            brep = cpool.tile([D, 384], F32, tag="brep")
            brep16 = cpool.tile([D, 128], F16, tag="brep16")

            for l in range(L):
                # ---- bias broadcast to all partitions (PE trick) ----
                bps = ppa.tile([D, 384], F32, tag="ps1")
                nc.tensor.matmul(
                    bps[:], lhsT=ones_row[:], rhs=bp_sb[:, l * 384 : (l + 1) * 384],
                    start=True, stop=True,
                )
                nc.vector.tensor_copy(brep[:], bps[:])
                if l == 0:
                    nc.vector.tensor_copy(brep16[:], bps[:, 256:384])
                nc.vector.memset(er_stage[:], 0.0)

                # ---- P1: feat/el table build (lo half = cores 0..3 first) ----
                wcl = wc_sb[:, l * (HF + 4) : l * (HF + 4) + HF + 4]
                scope = nc.named_scope(f"p1_l{l}")
                scope.__enter__()
                for cb in range(C):
                    tb = tb_lo if cb < 4 else tb_hi
                    nb0 = cb * cfg.nloc - (0 if cb < 4 else NLO)
                    for bt in range(0, T, P1B):
                        ntl = min(P1B, T - bt)
                        # partial last tile handled separately (store shape)
                        full = ntl if bt + ntl < T else ntl - 1
                        w_tot = full * 128 + (
                            0 if bt + ntl < T else cfg.w_last
                        )
                        xt = pool.tile([D, P1B * 128], F16, tag="xt")
                        if l == 0:
                            src_ap = xTb[cb, :, bt * 128 : bt * 128 + w_tot]
                        else:
                            src_ap = hT_all[cb, :, bt * 128 : bt * 128 + w_tot]
                        nc.sync.dma_start(out=xt[:, :w_tot], in_=src_ap)
                        stage = pool.tile([D, P1B, ROW16], F16, tag="stage")
                        for j in range(ntl):
                            w = 128 if bt + j < T - 1 else cfg.w_last
                            ps1 = ppa.tile([D, 384], F32, tag="ps1")
                            nc.tensor.matmul(
                                ps1[:w, 0 : HF + 4],
                                lhsT=xt[:, j * 128 : j * 128 + w],
                                rhs=wcl, start=True, stop=True,
                            )
                            # PSUM->SBUF copies alternate DVE/ACT (GPSIMD
                            # cannot read PSUM)
                            if j % 2 == 0:
                                nc.vector.tensor_copy(
                                    stage[:w, j, 0:HF], ps1[:w, 0:HF]
                                )
                                nc.vector.tensor_copy(
                                    stage[:w, j, HF : HF + 4].bitcast(F32),
                                    ps1[:w, HF : HF + 2],
                                )
                            else:
                                nc.scalar.activation(
                                    stage[:w, j, 0:HF], ps1[:w, 0:HF],
                                    mybir.ActivationFunctionType.Copy,
                                )
                                nc.scalar.activation(
                                    stage[:w, j, HF : HF + 4].bitcast(F32),
                                    ps1[:w, HF : HF + 2],
                                    mybir.ActivationFunctionType.Copy,
                                )
                            erv = er_stage[:, :, :].rearrange(
                                "p t (h k) -> p t h k", k=8
                            )
                            nc.vector.tensor_copy(
                                erv[:w, bt + j, :, cb], ps1[:w, HF + 2 : HF + 4]
                            )
                        # batched store of the full tiles, partial tile alone
                        if full > 0:
                            n0 = nb0 + bt * 128
                            nc.sync.dma_start(
                                out=tb[n0 : n0 + full * 128, :].rearrange(
                                    "(j p) e -> p j e", p=128
                                ),
                                in_=stage[:, 0:full, :],
                            )
                        if full < ntl:
                            n0 = nb0 + (bt + full) * 128
                            nc.sync.dma_start(
                                out=tb[n0 : n0 + cfg.w_last, :],
                                in_=stage[: cfg.w_last, full, :],
                            )

                # er_own = own core's column of er_stage
                er4 = er_stage[:, :, :].rearrange("p t (h k) -> p t h k", k=8)
                tmp_er = pool.tile([D, T, 2, 8], F32, tag="tmp_er")
                nc.vector.tensor_tensor(
                    out=tmp_er[:],
                    in0=er4,
                    in1=oh_sb[:].unsqueeze(1).unsqueeze(1).to_broadcast(
                        [D, T, 2, 8]
                    ),
                    op=mybir.AluOpType.mult,
                )
                nc.vector.reduce_sum(
                    er_own[:], tmp_er[:], axis=mybir.AxisListType.X
                )
                # er table for the per-edge dst gather (row j = own node j)
                nc.sync.dma_start(
                    out=tsmall[:, :].rearrange("(t p) v -> p t v", p=128)[
                        :, :, 0:2
                    ],
                    in_=er_own[:],
                )
                scope.__exit__(None, None, None)
                scope = nc.named_scope(f"p2_l{l}")
                scope.__enter__()

                # ---- P2: edge phase, batched gathers ----
                for bi, (t0, ntl) in enumerate(batches):
                    g_lo0, g_hi0, g_end = binfo[bi]
                    n_lo = g_hi0 - g_lo0
                    n_all = g_end - g_lo0

                    gt = pool.tile([D, GB_MAX, ROW16], F16, tag="gt")
                    nc.gpsimd.dma_gather(
                        out_ap=gt[:, 0:n_lo, :],
                        in_ap=tb_lo[:, :],
                        idxs_ap=src_sb[:, g_lo0 * 8 : g_hi0 * 8],
                        num_idxs=n_lo * 128,
                        num_idxs_reg=n_lo * 128,
                        elem_size=ROW16,
                        queue_num=0,
                        single_packet=False,
                    )
                    nc.gpsimd.dma_gather(
                        out_ap=gt[:, n_lo:n_all, :],
                        in_ap=tb_hi[:, :],
                        idxs_ap=src_sb[:, g_hi0 * 8 : g_end * 8],
                        num_idxs=(n_all - n_lo) * 128,
                        num_idxs_reg=(n_all - n_lo) * 128,
                        elem_size=ROW16,
                        queue_num=1,
                        single_packet=False,
                    )
                    ert = pool.tile([D, GB_MAX, 64], F32, tag="ert")
                    nc.gpsimd.dma_gather(
                        out_ap=ert[:, 0:n_all, :],
                        in_ap=tsmall[:, :],
                        idxs_ap=er_sb[:, g_lo0 * 8 : g_end * 8],
                        num_idxs=n_all * 128,
                        num_idxs_reg=n_all * 128,
                        elem_size=64,
                        queue_num=0,
                        single_packet=False,
                    )

                    for j in range(ntl):
                        t = t0 + j
                        w = 128 if t < T - 1 else cfg.w_last
                        gl, gh = int(G[t, 0]), int(G[t, 1])
                        gt_n = gl + gh
                        # (batch-local group start, count, tile-local start)
                        rngs = [
                            (int(base[t, 0]) - g_lo0, gl, 0),
                            (int(base[t, 1]) - g_lo0, gh, gl),
                        ]

                        S = pool.tile([D, GT_MAX, D], F16, tag="S")
                        msg = pool.tile([D, GT_MAX, 258], F16, tag="msg")
                        ere = pool.tile([D, GT_MAX, 2], F32, tag="ere")
                        lr = pool.tile([D, GT_MAX, 2], F32, tag="lr")

                        for (bg0, cnt, k0) in rngs:
                            ga = g_lo0 + bg0  # absolute group index
                            nc.sync.dma_start(
                                out=S[:, k0 : k0 + cnt, :],
                                in_=s_d[:, ga * 128 : (ga + cnt) * 128]
                                .rearrange("p (g d) -> p g d", d=128),
                            )
                            elv = gt[:, bg0 : bg0 + cnt, HF : HF + 4].bitcast(
                                F32
                            )
                            nc.vector.tensor_tensor(
                                out=ere[:, k0 : k0 + cnt, :],
                                in0=ert[:, bg0 : bg0 + cnt, 0:2],
                                in1=elv,
                                op=mybir.AluOpType.add,
                            )

                        # w = exp(lrelu(u)); exp on the Scalar engine, written
                        # directly into msg's denominator columns
                        nc.vector.tensor_scalar_mul(
                            lr[:, 0:gt_n, :], ere[:, 0:gt_n, :], NEG_SLOPE
                        )
                        nc.vector.tensor_tensor(
                            out=ere[:, 0:gt_n, :], in0=ere[:, 0:gt_n, :],
                            in1=lr[:, 0:gt_n, :], op=mybir.AluOpType.max,
                        )
                        nc.scalar.activation(
                            msg[:, 0:gt_n, 256:258], ere[:, 0:gt_n, :],
                            mybir.ActivationFunctionType.Exp,
                        )

                        for (bg0, cnt, k0) in rngs:
                            nc.vector.tensor_tensor(
                                out=msg[:, k0 : k0 + cnt, 0:256].rearrange(
                                    "p g (h f) -> p g h f", h=2
                                ),
                                in0=gt[:, bg0 : bg0 + cnt, 0:256].rearrange(
                                    "p g (h f) -> p g h f", h=2
                                ),
                                in1=msg[:, k0 : k0 + cnt, 256:258]
                                .unsqueeze(3).to_broadcast([D, cnt, 2, 128]),
                                op=mybir.AluOpType.mult,
                            )

                        ps2 = ppb.tile([D, 258], F32, tag="ps2")
                        for k in range(gt_n):
                            nc.tensor.matmul(
                                ps2[:],
                                lhsT=S[:, k, :],
                                rhs=msg[:, k, :],
                                start=(k == 0),
                                stop=(k == gt_n - 1),
                            )

                        # ---- finalize ----
                        osb = pool.tile([D, 258], F32, tag="osb")
                        nc.vector.tensor_copy(osb[:], ps2[:])
                        if l == 0:
                            # h = 0.5*(n0*r0 + n1*r1) + bmean, transposed;
                            # all math on the SBUF copy (PSUM reads are slow)
                            rsb = pool.tile([D, 2], F32, tag="rsb")
                            nc.vector.tensor_scalar_max(
                                rsb[:], osb[:, 256:258], 1e-30
                            )
                            nc.vector.reciprocal(rsb[:], rsb[:])
                            rh = pool.tile([D, 2], F32, tag="rh")
                            nc.vector.tensor_scalar_mul(rh[:], rsb[:], 0.5)
                            t_0 = pool.tile([D, D], F16, tag="t0")
                            nc.vector.tensor_scalar_mul(
                                t_0[:], osb[:, 0:128], rh[:, 0:1]
                            )
                            t_1 = pool.tile([D, D], F16, tag="t1")
                            nc.vector.tensor_scalar_mul(
                                t_1[:], osb[:, 128:256], rh[:, 1:2]
                            )
                            nc.vector.tensor_tensor(
                                out=t_0[:], in0=t_0[:], in1=t_1[:],
                                op=mybir.AluOpType.add,
                            )
                            ht16 = pool.tile([D, D], F16, tag="ht16")
                            nc.vector.tensor_tensor(
                                out=ht16[:], in0=t_0[:], in1=brep16[:],
                                op=mybir.AluOpType.add,
                            )
                            pst = ppa.tile([D, 384], F32, tag="ps1")
                            pst16 = pst.bitcast(F16)
                            nc.tensor.transpose(
                                pst16[:, 0:128], ht16[:], ident[:]
                            )
                            nc.vector.tensor_copy(
                                hT_stage[:, t * 128 : (t + 1) * 128],
                                pst16[:, 0:128],
                            )
                        else:
                            # raw sums out; host normalizes and adds bias
                            nc.sync.dma_start(
                                out=out_d[t * 128 : (t + 1) * 128, :], in_=osb[:]
                            )

                scope.__exit__(None, None, None)
                # ---- inter-layer allgather ----
                if l == 0:
                    nc.sync.dma_start(
                        out=hT_own[:], in_=hT_stage[:, 0 : cfg.nloc]
                    )
                    with nc.named_scope("cc"):
                        nc.gpsimd.collective_compute(
                            "AllGather",
                            mybir.AluOpType.bypass,
                            replica_groups=[list(range(C))],
                            ins=[hT_own[:]],
                            outs=[hT_all[:]],
                        )
    nc.compile()
    return nc


# ----------------------------------------------------------------------------
# Entry point
# ----------------------------------------------------------------------------

def run_gat(cfg, x, Ws, als, ars, bs, src, dst, trace=False):
    geom, src_w, er_w, s_tab = prep_edges(cfg, src, dst)
    wc, bp = prep_weights(cfg, Ws, als, ars, bs)

    x = np.asarray(x, dtype=np.float32)
    xTb = np.ascontiguousarray(
        x.reshape(N_CORES, cfg.nloc, D).transpose(0, 2, 1)
    ).astype(np.float16)

    onehots = []
    for c in range(N_CORES):
        oh = np.zeros((D, 8), dtype=np.float32)
        oh[:, c] = 1.0
        onehots.append(oh)

    nc = build(cfg, geom)
    in_maps = []
    for c in range(N_CORES):
        in_maps.append({
            "xTb": xTb,
            "wc": wc,
            "bp": bp,
            "srcw": src_w[c],
            "erw": er_w[c],
            "s_tab": s_tab[c],
            "onehot": onehots[c],
        })
    res = run_bass_kernel_spmd(nc, in_maps, list(range(N_CORES)), trace=trace)
    outs = [res.results[c]["out"][: cfg.nloc] for c in range(N_CORES)]
    raw = np.concatenate(outs, axis=0).astype(np.float64)   # (n, 258)
    num = raw[:, 0:HF].reshape(cfg.n, H, D)
    den = np.maximum(raw[:, HF : HF + 2], 1e-30)            # (n, H)
    out = num / den[:, :, None] + np.asarray(bs[-1], np.float64)[None]
    return out.astype(np.float32), res


def kernel(x, Ws, als, ars, bs, src, dst):
    out, _ = run_gat(FULL, x, Ws, als, ars, bs, src, dst, trace=False)
    return out.astype(np.float32)


# revision 52
# speedup vs baseline: 1.3287x; 1.3287x over previous
"""Trainium2 Bass kernel for 2-layer GAT (nn_GAT_43765716746408).

Self-contained: hardcodes the problem geometry (50000 nodes, 800000 edges,
D=128, H=2 heads, F=128, 2 layers) and distributes across 8 NeuronCores by
dst-node partition.

Strategy per layer (SPMD across 8 cores, identical program, per-core data):
  - Replicated feature matmul (P1): every core computes feat = h @ [W|wl|wr]
    for ALL nodes (wl/wr fold the attention vectors al/ar into the matmul so
    el = feat@al, er = feat@ar come out as 4 extra columns), writing a packed
    row table T[n] = [feat fp16 (512B) | el f32 (8B) | pad] (768B rows),
    split into two DRAM tables (lo/hi node halves) so edge gathers on the lo
    half can start while the hi half is still being written.  er for the
    core's own nodes is kept in SBUF (er_own).
  - Edge phase (P2): edges are bucketed by (dst-tile-of-128, src-half) on the
    host (padded with dummy edges to uniform bucket sizes shared by all
    cores), and buckets are grouped into batches of a few dst tiles so each
    dma_gather instruction covers thousands of edges (SWDGE descriptor
    generation on GpSimd has a large per-instruction cost).  Per dst tile:
    one-hot S[e, d] = (dst[e] == d) via iota + is_equal, transposed one-hot
    S_T[d, e] via a host-replicated dst table + partition iota, er per edge
    via a small matmul (S_T^T @ er_own), w = exp(lrelu(el + er)) on the
    Scalar engine written directly into msg columns 256:258, msg = w * feat
    on Vector, and out[d] = sum_e S[e,d] * msg[e] via one 128x258 matmul per
    128-edge group accumulated in PSUM.  Finalize divides by the accumulated
    denominators (msg cols 256:258 aggregate to per-dst sums of w) and adds
    bias.
  - Between layers: h = mean over heads, transposed on-chip to (feat, node)
    layout and AllGather'd so every core has the full h for layer 2's
    replicated matmul.
"""

import sys

sys.path.insert(0, "/opt/trn_rl_repo")

import numpy as np

import concourse.bass as bass
import concourse.tile as tile
from concourse import bacc, mybir
from concourse.bass_utils import run_bass_kernel_spmd
from concourse.masks import make_identity

F32 = mybir.dt.float32
F16 = mybir.dt.float16
I16 = mybir.dt.int16

N_CORES = 8
D = 128          # model dim
H = 2            # heads
HF = 256         # H * F
ROW16 = 384      # fp16 elements per table row (512B feat + 8B el + pad = 768B)
NEG_SLOPE = 0.2
B_TILES = 2      # dst tiles per gather batch
P1B = 4          # node tiles per P1 load/store batch


class Cfg:
    def __init__(self, n_nodes, n_edges, n_layers=2):
        assert n_nodes % N_CORES == 0
        self.n = n_nodes
        self.e = n_edges
        self.layers = n_layers
        self.nloc = n_nodes // N_CORES
        self.t = -(-self.nloc // 128)          # dst tiles per core
        self.nloc_pad = self.t * 128
        self.w_last = self.nloc - 128 * (self.t - 1)
        self.split = n_nodes // 2              # lo/hi table split (int16 range)
        assert self.split < 32768 and (n_nodes - self.split) < 32768
        assert self.nloc_pad < 32768


FULL = Cfg(50000, 800000)


# ----------------------------------------------------------------------------
# Host-side edge preprocessing
# ----------------------------------------------------------------------------

def prep_edges(cfg, src, dst):
    """Bucket edges per core by (dst_tile, src_half); pad to shared sizes.

    Slot order is batch-major: for each batch of B_TILES dst tiles, first all
    lo buckets of the batch's tiles, then all hi buckets, so one dma_gather
    per (batch, half) covers a contiguous slot range.
    """
    C, T = N_CORES, cfg.t
    counts = np.zeros((C, T, 2), dtype=np.int64)
    per_core = []
    core_of = dst // cfg.nloc
    for c in range(C):
        sel = core_of == c
        es, ed = src[sel].astype(np.int64), dst[sel].astype(np.int64)
        dloc = ed - c * cfg.nloc
        t = dloc // 128
        half = (es >= cfg.split).astype(np.int64)
        # sort by (tile, half, src) for gather locality
        order = np.lexsort((es, half, t))
        es, dloc, t, half = es[order], dloc[order], t[order], half[order]
        np.add.at(counts[c], (t, half), 1)
        per_core.append((es, dloc, t, half))

    gmax_th = counts.max(axis=0)                       # (T, 2)
    G = np.maximum(1, -(-gmax_th // 128))              # groups per (t, half)

    batches = [(t0, min(B_TILES, T - t0)) for t0 in range(0, T, B_TILES)]
    base = np.zeros((T, 2), dtype=np.int64)            # group offset per bucket
    acc = 0
    binfo = []                                         # (g_lo0, g_hi0, g_end)
    for (t0, nt) in batches:
        g_lo0 = acc
        for t in range(t0, t0 + nt):
            base[t, 0] = acc
            acc += G[t, 0]
        g_hi0 = acc
        for t in range(t0, t0 + nt):
            base[t, 1] = acc
            acc += G[t, 1]
        binfo.append((g_lo0, g_hi0, acc))
    gtot = acc
    nslot = gtot * 128

    src_idx = np.zeros((C, nslot), dtype=np.int16)
    dst_reb = np.full((C, nslot), -1.0, dtype=np.float16)
    for c in range(C):
        es, dloc, t, half = per_core[c]
        # edges are lexsorted by (t, half, src) so buckets are contiguous:
        # position within bucket = arange - bucket start
        bucket_id = t * 2 + half
        n = len(es)
        starts = np.searchsorted(bucket_id, np.arange(T * 2), side="left")
        pos_in_bucket = np.arange(n) - starts[bucket_id]
        slot = base[t, half] * 128 + pos_in_bucket
        src_idx[c, slot] = (es - np.where(half == 1, cfg.split, 0)).astype(np.int16)
        dst_reb[c, slot] = (dloc - t * 128).astype(np.float32)

    # wrapped int16 index layout: element s -> [s % 16, s // 16], replicated
    # to 128 partitions (the 8 gpsimd cores each read their 16-partition copy)
    def wrap16(a):
        w = a.reshape(-1, 16).T.copy()                 # (16, nslot/16)
        return np.tile(w, (8, 1))                      # (128, nslot/16)

    src_w = np.stack([wrap16(src_idx[c]) for c in range(C)])
    # host-precomputed one-hot matrices (layer-invariant, streamed from DRAM):
    # S[slot, d]  = (dst_reb[slot] == d)   rows=slot, for lhsT of aggregation
    # ST[d, slot] = (dst_reb[slot] == d)   rows=d, for lhsT of er broadcast
    dgrid = np.arange(D, dtype=np.float16)
    # S stored partition-contiguous: s_tab[e, g*128 + d] = (dst[g*128+e] == d)
    # so each device load is 128 large contiguous descriptors, not 128*cnt
    # small strided ones.
    s_tab = np.stack(
        [
            (dst_reb[c].reshape(gtot, 128)[:, :, None] == dgrid[None, None, :])
            .astype(np.float16).transpose(1, 0, 2).reshape(D, nslot)
            for c in range(C)
        ]
    )                                                  # (C, 128, nslot)
    st_tab = np.stack(
        [(dst_reb[c][None, :] == dgrid[:, None]).astype(np.float16)
         for c in range(C)]
    )                                                  # (C, 128, nslot)

    geom = {
        "G": G,
        "base": base,
        "gtot": gtot,
        "batches": batches,
        "binfo": binfo,
        "gt_max": int((G[:, 0] + G[:, 1]).max()),
        "gb_max": int(max(e - s for (s, _h, e) in binfo)),
    }
    return geom, src_w, s_tab, st_tab


def prep_weights(cfg, Ws, als, ars, bs):
    """Combined matmul weights Wc = [W | wl | wr] and packed bias rows.

    W[l] is (D, H*F) with head-major columns; wl[k,h] = sum_f W[k,h,f]*al[h,f]
    folds the attention dot products into the same matmul.
    """
    L = cfg.layers
    wc = np.zeros((L, D, HF + 4), dtype=np.float16)
    bp = np.zeros((L, 1, 384), dtype=np.float16)
    for l in range(L):
        W = np.asarray(Ws[l], dtype=np.float32)            # (D, H*F)
        Wh = W.reshape(D, H, D)                            # (D, H, F)
        wl = np.einsum("khf,hf->kh", Wh, np.asarray(als[l], np.float32))
        wr = np.einsum("khf,hf->kh", Wh, np.asarray(ars[l], np.float32))
        wc[l, :, :HF] = W.astype(np.float16)
        wc[l, :, HF : HF + 2] = wl.astype(np.float16)
        wc[l, :, HF + 2 : HF + 4] = wr.astype(np.float16)
        b = np.asarray(bs[l], np.float32)                  # (H, F)
        bp[l, 0, 0:128] = b[0].astype(np.float16)
        bp[l, 0, 128:256] = b[1].astype(np.float16)
        bp[l, 0, 256:384] = (0.5 * (b[0] + b[1])).astype(np.float16)
    return wc, bp


# ----------------------------------------------------------------------------
# Device kernel
# ----------------------------------------------------------------------------

def build(cfg, geom):
    C, T, L = N_CORES, cfg.t, cfg.layers
    G, base = geom["G"], geom["base"]
    gtot, batches, binfo = geom["gtot"], geom["batches"], geom["binfo"]
    GT_MAX, GB_MAX = geom["gt_max"], geom["gb_max"]
    nslot = gtot * 128
    NLO, NHI = cfg.split, cfg.n - cfg.split

    nc = bacc.Bacc("TRN2", target_bir_lowering=False, debug=False,
                   enable_asserts=False, num_devices=C, num_swdge_queues=2,
                   dynamic_dma_scratch_size=32768)

    # I/O
    xTb = nc.dram_tensor("xTb", [C, D, cfg.nloc], F16, kind="ExternalInput")
    wc_d = nc.dram_tensor("wc", [L, D, HF + 4], F16, kind="ExternalInput")
    bp_d = nc.dram_tensor("bp", [L, 1, 384], F16, kind="ExternalInput")
    src_d = nc.dram_tensor("srcw", [D, nslot // 16], I16, kind="ExternalInput")
    s_d = nc.dram_tensor("s_tab", [D, nslot], F16, kind="ExternalInput")
    st_d = nc.dram_tensor("st_tab", [D, nslot], F16, kind="ExternalInput")
    oh_d = nc.dram_tensor("onehot", [D, 8], F32, kind="ExternalInput")
    # raw layer-2 accumulators: [num_h0 | num_h1 | den_h0 | den_h1];
    # normalization + bias happen on the host
    out_d = nc.dram_tensor("out", [cfg.nloc_pad, 258], F32, kind="ExternalOutput")

    # internal DRAM
    tb_lo = nc.dram_tensor("tb_lo", [NLO, ROW16], F16)
    tb_hi = nc.dram_tensor("tb_hi", [NHI, ROW16], F16)
    hT_own = nc.dram_tensor("hT_own", [D, cfg.nloc], F16)
    hT_all = nc.dram_tensor("hT_all", [C, D, cfg.nloc], F16, addr_space="Shared")

    assert NLO == 4 * cfg.nloc, "lo half must be cores 0..3"

    with tile.TileContext(nc) as tc:
        with (
            tc.tile_pool(name="const", bufs=1) as cpool,
            tc.tile_pool(name="work", bufs=2) as pool,
            tc.tile_pool(name="ps_a", bufs=3, space="PSUM") as ppa,
            tc.tile_pool(name="ps_b", bufs=2, space="PSUM") as ppb,
            tc.tile_pool(name="ps_c", bufs=2, space="PSUM") as ppc,
        ):
            # ---- constants ----
            src_sb = cpool.tile([D, nslot // 16], I16, tag="src_sb")
            nc.sync.dma_start(out=src_sb[:], in_=src_d[:])
            oh_sb = cpool.tile([D, 8], F32, tag="oh_sb")
            nc.sync.dma_start(out=oh_sb[:], in_=oh_d[:])
            wc_sb = cpool.tile([D, L * (HF + 4)], F16, tag="wc_sb")
            bp_sb = cpool.tile([1, L * 384], F16, tag="bp_sb")
            for l in range(L):
                nc.sync.dma_start(
                    out=wc_sb[:, l * (HF + 4) : (l + 1) * (HF + 4)], in_=wc_d[l]
                )
                nc.sync.dma_start(
                    out=bp_sb[:, l * 384 : (l + 1) * 384], in_=bp_d[l]
                )

            ones_row = cpool.tile([1, D], F16, tag="ones_row")
            nc.vector.memset(ones_row[:], 1.0)
            ident = cpool.tile([D, D], F16, tag="ident")
            make_identity(nc, ident[:])

            er_stage = cpool.tile([D, T, 16], F32, tag="er_stage")
            er_own = cpool.tile([D, T, 2], F32, tag="er_own")
            er_own16 = cpool.tile([D, T, 2], F16, tag="er_own16")
            hT_stage = cpool.tile([D, cfg.nloc_pad], F16, tag="hT_stage")
            brep = cpool.tile([D, 384], F32, tag="brep")
            brep16 = cpool.tile([D, 128], F16, tag="brep16")

            for l in range(L):
                # ---- bias broadcast to all partitions (PE trick) ----
                bps = ppa.tile([D, 384], F32, tag="ps1")
                nc.tensor.matmul(
                    bps[:], lhsT=ones_row[:], rhs=bp_sb[:, l * 384 : (l + 1) * 384],
                    start=True, stop=True,
                )
                nc.vector.tensor_copy(brep[:], bps[:])
                if l == 0:
                    nc.vector.tensor_copy(brep16[:], bps[:, 256:384])
                nc.vector.memset(er_stage[:], 0.0)

                # ---- P1: feat/el table build (lo half = cores 0..3 first) ----
                wcl = wc_sb[:, l * (HF + 4) : l * (HF + 4) + HF + 4]
                scope = nc.named_scope(f"p1_l{l}")
                scope.__enter__()
                for cb in range(C):
                    tb = tb_lo if cb < 4 else tb_hi
                    nb0 = cb * cfg.nloc - (0 if cb < 4 else NLO)
                    for bt in range(0, T, P1B):
                        ntl = min(P1B, T - bt)
                        # partial last tile handled separately (store shape)
                        full = ntl if bt + ntl < T else ntl - 1
                        w_tot = full * 128 + (
                            0 if bt + ntl < T else cfg.w_last
                        )
                        xt = pool.tile([D, P1B * 128], F16, tag="xt")
                        if l == 0:
                            src_ap = xTb[cb, :, bt * 128 : bt * 128 + w_tot]
                        else:
                            src_ap = hT_all[cb, :, bt * 128 : bt * 128 + w_tot]
                        nc.sync.dma_start(out=xt[:, :w_tot], in_=src_ap)
                        stage = pool.tile([D, P1B, ROW16], F16, tag="stage")
                        for j in range(ntl):
                            w = 128 if bt + j < T - 1 else cfg.w_last
                            ps1 = ppa.tile([D, 384], F32, tag="ps1")
                            nc.tensor.matmul(
                                ps1[:w, 0 : HF + 4],
                                lhsT=xt[:, j * 128 : j * 128 + w],
                                rhs=wcl, start=True, stop=True,
                            )
                            # PSUM->SBUF copies alternate DVE/ACT (GPSIMD
                            # cannot read PSUM)
                            if j % 2 == 0:
                                nc.vector.tensor_copy(
                                    stage[:w, j, 0:HF], ps1[:w, 0:HF]
                                )
                                nc.vector.tensor_copy(
                                    stage[:w, j, HF : HF + 4].bitcast(F32),
                                    ps1[:w, HF : HF + 2],
                                )
                            else:
                                nc.scalar.activation(
                                    stage[:w, j, 0:HF], ps1[:w, 0:HF],
                                    mybir.ActivationFunctionType.Copy,
                                )
                                nc.scalar.activation(
                                    stage[:w, j, HF : HF + 4].bitcast(F32),
                                    ps1[:w, HF : HF + 2],
                                    mybir.ActivationFunctionType.Copy,
                                )
                            erv = er_stage[:, :, :].rearrange(
                                "p t (h k) -> p t h k", k=8
                            )
                            nc.vector.tensor_copy(
                                erv[:w, bt + j, :, cb], ps1[:w, HF + 2 : HF + 4]
                            )
                        # batched store of the full tiles, partial tile alone
                        if full > 0:
                            n0 = nb0 + bt * 128
                            nc.sync.dma_start(
                                out=tb[n0 : n0 + full * 128, :].rearrange(
                                    "(j p) e -> p j e", p=128
                                ),
                                in_=stage[:, 0:full, :],
                            )
                        if full < ntl:
                            n0 = nb0 + (bt + full) * 128
                            nc.sync.dma_start(
                                out=tb[n0 : n0 + cfg.w_last, :],
                                in_=stage[: cfg.w_last, full, :],
                            )

                # er_own = own core's column of er_stage
                er4 = er_stage[:, :, :].rearrange("p t (h k) -> p t h k", k=8)
                tmp_er = pool.tile([D, T, 2, 8], F32, tag="tmp_er")
                nc.vector.tensor_tensor(
                    out=tmp_er[:],
                    in0=er4,
                    in1=oh_sb[:].unsqueeze(1).unsqueeze(1).to_broadcast(
                        [D, T, 2, 8]
                    ),
                    op=mybir.AluOpType.mult,
                )
                nc.vector.reduce_sum(
                    er_own[:], tmp_er[:], axis=mybir.AxisListType.X
                )
                nc.vector.tensor_copy(er_own16[:], er_own[:])
                scope.__exit__(None, None, None)
                scope = nc.named_scope(f"p2_l{l}")
                scope.__enter__()

                # ---- P2: edge phase, batched gathers; the tile loop is
                # software-pipelined (finalize deferred two tiles) so finalize
                # ops that depend on the aggregation don't head-of-line block
                # the next tile's ready work in the in-order engine queues.
                def emit_pre(bi, t, gt):
                    g_lo0 = binfo[bi][0]
                    gl, gh = int(G[t, 0]), int(G[t, 1])
                    gt_n = gl + gh
                    rngs = [
                        (int(base[t, 0]) - g_lo0, gl, 0),
                        (int(base[t, 1]) - g_lo0, gh, gl),
                    ]
                    S = pool.tile([D, GT_MAX, D], F16, tag="S", bufs=3)
                    ST = pool.tile([D, GT_MAX, D], F16, tag="ST", bufs=3)
                    msg = pool.tile([D, GT_MAX, 258], F16, tag="msg", bufs=3)
                    ere = pool.tile([D, GT_MAX, 2], F32, tag="ere", bufs=3)
                    lr = pool.tile([D, GT_MAX, 2], F32, tag="lr", bufs=3)
                    erp = ppc.tile([D, GT_MAX, 2], F32, tag="erp")
                    for (bg0, cnt, k0) in rngs:
                        ga = g_lo0 + bg0  # absolute group index
                        nc.sync.dma_start(
                            out=S[:, k0 : k0 + cnt, :],
                            in_=s_d[:, ga * 128 : (ga + cnt) * 128]
                            .rearrange("p (g d) -> p g d", d=128),
                        )
                        nc.sync.dma_start(
                            out=ST[:, k0 : k0 + cnt, :],
                            in_=st_d[:, ga * 128 : (ga + cnt) * 128]
                            .rearrange("p (g e) -> p g e", e=128),
                        )
                        # er per edge: one small matmul per group
                        for k in range(cnt):
                            nc.tensor.matmul(
                                erp[:, k0 + k, :],
                                lhsT=ST[:, k0 + k, :],
                                rhs=er_own16[:, t, :],
                                start=True, stop=True,
                            )
                        elv = gt[:, bg0 : bg0 + cnt, HF : HF + 4].bitcast(F32)
                        nc.vector.tensor_tensor(
                            out=ere[:, k0 : k0 + cnt, :],
                            in0=erp[:, k0 : k0 + cnt, :],
                            in1=elv,
                            op=mybir.AluOpType.add,
                        )
                    # w = exp(lrelu(u)); exp on the Scalar engine, written
                    # directly into msg's denominator columns
                    nc.vector.tensor_scalar_mul(
                        lr[:, 0:gt_n, :], ere[:, 0:gt_n, :], NEG_SLOPE
                    )
                    nc.vector.tensor_tensor(
                        out=ere[:, 0:gt_n, :], in0=ere[:, 0:gt_n, :],
                        in1=lr[:, 0:gt_n, :], op=mybir.AluOpType.max,
                    )
                    nc.scalar.activation(
                        msg[:, 0:gt_n, 256:258], ere[:, 0:gt_n, :],
                        mybir.ActivationFunctionType.Exp,
                    )
                    for (bg0, cnt, k0) in rngs:
                        nc.vector.tensor_tensor(
                            out=msg[:, k0 : k0 + cnt, 0:256].rearrange(
                                "p g (h f) -> p g h f", h=2
                            ),
                            in0=gt[:, bg0 : bg0 + cnt, 0:256].rearrange(
                                "p g (h f) -> p g h f", h=2
                            ),
                            in1=msg[:, k0 : k0 + cnt, 256:258]
                            .unsqueeze(3).to_broadcast([D, cnt, 2, 128]),
                            op=mybir.AluOpType.mult,
                        )
                    ps2 = ppb.tile([D, 258], F32, tag="ps2", bufs=3)
                    for k in range(gt_n):
                        nc.tensor.matmul(
                            ps2[:],
                            lhsT=S[:, k, :],
                            rhs=msg[:, k, :],
                            start=(k == 0),
                            stop=(k == gt_n - 1),
                        )
                    return ps2

                def emit_fin(t, ps2):
                    osb = pool.tile([D, 258], F32, tag="osb")
                    nc.vector.tensor_copy(osb[:], ps2[:])
                    if l == 0:
                        # h = 0.5*(n0*r0 + n1*r1) + bmean, transposed;
                        # all math on the SBUF copy (PSUM reads are slow)
                        rsb = pool.tile([D, 2], F32, tag="rsb")
                        nc.vector.tensor_scalar_max(
                            rsb[:], osb[:, 256:258], 1e-30
                        )
                        nc.vector.reciprocal(rsb[:], rsb[:])
                        rh = pool.tile([D, 2], F32, tag="rh")
                        nc.vector.tensor_scalar_mul(rh[:], rsb[:], 0.5)
                        t_0 = pool.tile([D, D], F16, tag="t0")
                        nc.vector.tensor_scalar_mul(
                            t_0[:], osb[:, 0:128], rh[:, 0:1]
                        )
                        t_1 = pool.tile([D, D], F16, tag="t1")
                        nc.vector.tensor_scalar_mul(
                            t_1[:], osb[:, 128:256], rh[:, 1:2]
                        )
                        nc.vector.tensor_tensor(
                            out=t_0[:], in0=t_0[:], in1=t_1[:],
                            op=mybir.AluOpType.add,
                        )
                        ht16 = pool.tile([D, D], F16, tag="ht16")
                        nc.vector.tensor_tensor(
                            out=ht16[:], in0=t_0[:], in1=brep16[:],
                            op=mybir.AluOpType.add,
                        )
                        pst = ppa.tile([D, 384], F32, tag="ps1")
                        pst16 = pst.bitcast(F16)
                        nc.tensor.transpose(pst16[:, 0:128], ht16[:], ident[:])
                        nc.vector.tensor_copy(
                            hT_stage[:, t * 128 : (t + 1) * 128],
                            pst16[:, 0:128],
                        )
                    else:
                        # raw sums out; host normalizes and adds bias
                        nc.sync.dma_start(
                            out=out_d[t * 128 : (t + 1) * 128, :], in_=osb[:]
                        )

                pending = []
                cur_bi = -1
                gt = None
                for bi, (t0, ntl) in enumerate(batches):
                    for j in range(ntl):
                        t = t0 + j
                        if bi != cur_bi:
                            g_lo0, g_hi0, g_end = binfo[bi]
                            n_lo = g_hi0 - g_lo0
                            n_all = g_end - g_lo0
                            gt = pool.tile([D, GB_MAX, ROW16], F16, tag="gt")
                            nc.gpsimd.dma_gather(
                                out_ap=gt[:, 0:n_lo, :],
                                in_ap=tb_lo[:, :],
                                idxs_ap=src_sb[:, g_lo0 * 8 : g_hi0 * 8],
                                num_idxs=n_lo * 128,
                                num_idxs_reg=n_lo * 128,
                                elem_size=ROW16,
                                queue_num=0,
                                single_packet=False,
                            )
                            nc.gpsimd.dma_gather(
                                out_ap=gt[:, n_lo:n_all, :],
                                in_ap=tb_hi[:, :],
                                idxs_ap=src_sb[:, g_hi0 * 8 : g_end * 8],
                                num_idxs=(n_all - n_lo) * 128,
                                num_idxs_reg=(n_all - n_lo) * 128,
                                elem_size=ROW16,
                                queue_num=1,
                                single_packet=False,
                            )
                            cur_bi = bi
                        ps2 = emit_pre(bi, t, gt)
                        pending.append((t, ps2))
                        if len(pending) > 2:
                            emit_fin(*pending.pop(0))
                for p in pending:
                    emit_fin(*p)

                scope.__exit__(None, None, None)
                # ---- inter-layer allgather ----
                if l == 0:
                    nc.sync.dma_start(
                        out=hT_own[:], in_=hT_stage[:, 0 : cfg.nloc]
                    )
                    with nc.named_scope("cc"):
                        nc.gpsimd.collective_compute(
                            "AllGather",
                            mybir.AluOpType.bypass,
                            replica_groups=[list(range(C))],
                            ins=[hT_own[:]],
                            outs=[hT_all[:]],
                        )
    nc.compile()
    return nc


# ----------------------------------------------------------------------------
# Entry point
# ----------------------------------------------------------------------------

def run_gat(cfg, x, Ws, als, ars, bs, src, dst, trace=False):
    geom, src_w, s_tab, st_tab = prep_edges(cfg, src, dst)
    wc, bp = prep_weights(cfg, Ws, als, ars, bs)

    x = np.asarray(x, dtype=np.float32)
    xTb = np.ascontiguousarray(
        x.reshape(N_CORES, cfg.nloc, D).transpose(0, 2, 1)
    ).astype(np.float16)

    onehots = []
    for c in range(N_CORES):
        oh = np.zeros((D, 8), dtype=np.float32)
        oh[:, c] = 1.0
        onehots.append(oh)

    nc = build(cfg, geom)
    in_maps = []
    for c in range(N_CORES):
        in_maps.append({
            "xTb": xTb,
            "wc": wc,
            "bp": bp,
            "srcw": src_w[c],
            "s_tab": s_tab[c],
            "st_tab": st_tab[c],
            "onehot": onehots[c],
        })
    res = run_bass_kernel_spmd(nc, in_maps, list(range(N_CORES)), trace=trace)
    outs = [res.results[c]["out"][: cfg.nloc] for c in range(N_CORES)]
    raw = np.concatenate(outs, axis=0).astype(np.float64)   # (n, 258)
    num = raw[:, 0:HF].reshape(cfg.n, H, D)
    den = np.maximum(raw[:, HF : HF + 2], 1e-30)            # (n, H)
    out = num / den[:, :, None] + np.asarray(bs[-1], np.float64)[None]
    return out.astype(np.float32), res


def kernel(x, Ws, als, ars, bs, src, dst):
    out, _ = run_gat(FULL, x, Ws, als, ars, bs, src, dst, trace=False)
    return out.astype(np.float32)


# revision 54
# speedup vs baseline: 1.4310x; 1.0770x over previous
"""Trainium2 Bass kernel for 2-layer GAT (nn_GAT_43765716746408).

Self-contained: hardcodes the problem geometry (50000 nodes, 800000 edges,
D=128, H=2 heads, F=128, 2 layers) and distributes across 8 NeuronCores by
dst-node partition.

Strategy per layer (SPMD across 8 cores, identical program, per-core data):
  - Replicated feature matmul (P1): every core computes feat = h @ [W|wl|wr]
    for ALL nodes (wl/wr fold the attention vectors al/ar into the matmul so
    el = feat@al, er = feat@ar come out as 4 extra columns), writing a packed
    row table T[n] = [feat fp16 (512B) | el f32 (8B) | pad] (768B rows),
    split into two DRAM tables (lo/hi node halves) so edge gathers on the lo
    half can start while the hi half is still being written.  er for the
    core's own nodes is kept in SBUF (er_own).
  - Edge phase (P2): edges are bucketed by (dst-tile-of-128, src-half) on the
    host (padded with dummy edges to uniform bucket sizes shared by all
    cores), and buckets are grouped into batches of a few dst tiles so each
    dma_gather instruction covers thousands of edges (SWDGE descriptor
    generation on GpSimd has a large per-instruction cost).  Per dst tile:
    one-hot S[e, d] = (dst[e] == d) via iota + is_equal, transposed one-hot
    S_T[d, e] via a host-replicated dst table + partition iota, er per edge
    via a small matmul (S_T^T @ er_own), w = exp(lrelu(el + er)) on the
    Scalar engine written directly into msg columns 256:258, msg = w * feat
    on Vector, and out[d] = sum_e S[e,d] * msg[e] via one 128x258 matmul per
    128-edge group accumulated in PSUM.  Finalize divides by the accumulated
    denominators (msg cols 256:258 aggregate to per-dst sums of w) and adds
    bias.
  - Between layers: h = mean over heads, transposed on-chip to (feat, node)
    layout and AllGather'd so every core has the full h for layer 2's
    replicated matmul.
"""

import sys

sys.path.insert(0, "/opt/trn_rl_repo")

import numpy as np

import concourse.bass as bass
import concourse.tile as tile
from concourse import bacc, mybir
from concourse.bass_utils import run_bass_kernel_spmd
from concourse.masks import make_identity

F32 = mybir.dt.float32
F16 = mybir.dt.float16
I16 = mybir.dt.int16

N_CORES = 8
D = 128          # model dim
H = 2            # heads
HF = 256         # H * F
ROW16 = 384      # fp16 elements per table row (512B feat + 8B el + pad = 768B)
NEG_SLOPE = 0.2
B_TILES = 2      # dst tiles per gather batch
P1B = 4          # node tiles per P1 load/store batch


class Cfg:
    def __init__(self, n_nodes, n_edges, n_layers=2):
        assert n_nodes % N_CORES == 0
        self.n = n_nodes
        self.e = n_edges
        self.layers = n_layers
        self.nloc = n_nodes // N_CORES
        self.t = -(-self.nloc // 128)          # dst tiles per core
        self.nloc_pad = self.t * 128
        self.w_last = self.nloc - 128 * (self.t - 1)
        self.split = n_nodes // 2              # lo/hi table split (int16 range)
        assert self.split < 32768 and (n_nodes - self.split) < 32768
        assert self.nloc_pad < 32768


FULL = Cfg(50000, 800000)


# ----------------------------------------------------------------------------
# Host-side edge preprocessing
# ----------------------------------------------------------------------------

def prep_edges(cfg, src, dst):
    """Bucket edges per core by (dst_tile, src_half); pad to shared sizes.

    Slot order is batch-major: for each batch of B_TILES dst tiles, first all
    lo buckets of the batch's tiles, then all hi buckets, so one dma_gather
    per (batch, half) covers a contiguous slot range.
    """
    C, T = N_CORES, cfg.t
    counts = np.zeros((C, T, 2), dtype=np.int64)
    per_core = []
    core_of = dst // cfg.nloc
    for c in range(C):
        sel = core_of == c
        es, ed = src[sel].astype(np.int64), dst[sel].astype(np.int64)
        dloc = ed - c * cfg.nloc
        t = dloc // 128
        half = (es >= cfg.split).astype(np.int64)
        # sort by (tile, half, src) for gather locality
        order = np.lexsort((es, half, t))
        es, dloc, t, half = es[order], dloc[order], t[order], half[order]
        np.add.at(counts[c], (t, half), 1)
        per_core.append((es, dloc, t, half))

    gmax_th = counts.max(axis=0)                       # (T, 2)
    G = np.maximum(1, -(-gmax_th // 128))              # groups per (t, half)

    batches = [(t0, min(B_TILES, T - t0)) for t0 in range(0, T, B_TILES)]
    base = np.zeros((T, 2), dtype=np.int64)            # group offset per bucket
    acc = 0
    binfo = []                                         # (g_lo0, g_hi0, g_end)
    for (t0, nt) in batches:
        g_lo0 = acc
        for t in range(t0, t0 + nt):
            base[t, 0] = acc
            acc += G[t, 0]
        g_hi0 = acc
        for t in range(t0, t0 + nt):
            base[t, 1] = acc
            acc += G[t, 1]
        binfo.append((g_lo0, g_hi0, acc))
    gtot = acc
    nslot = gtot * 128

    src_idx = np.zeros((C, nslot), dtype=np.int16)
    dst_reb = np.full((C, nslot), -1.0, dtype=np.float16)
    for c in range(C):
        es, dloc, t, half = per_core[c]
        # edges are lexsorted by (t, half, src) so buckets are contiguous:
        # position within bucket = arange - bucket start
        bucket_id = t * 2 + half
        n = len(es)
        starts = np.searchsorted(bucket_id, np.arange(T * 2), side="left")
        pos_in_bucket = np.arange(n) - starts[bucket_id]
        slot = base[t, half] * 128 + pos_in_bucket
        src_idx[c, slot] = (es - np.where(half == 1, cfg.split, 0)).astype(np.int16)
        dst_reb[c, slot] = (dloc - t * 128).astype(np.float32)

    # wrapped int16 index layout: element s -> [s % 16, s // 16], replicated
    # to 128 partitions (the 8 gpsimd cores each read their 16-partition copy)
    def wrap16(a):
        w = a.reshape(-1, 16).T.copy()                 # (16, nslot/16)
        return np.tile(w, (8, 1))                      # (128, nslot/16)

    src_w = np.stack([wrap16(src_idx[c]) for c in range(C)])
    # host-precomputed one-hot matrices (layer-invariant, streamed from DRAM):
    # S[slot, d]  = (dst_reb[slot] == d)   rows=slot, for lhsT of aggregation
    # ST[d, slot] = (dst_reb[slot] == d)   rows=d, for lhsT of er broadcast
    dgrid = np.arange(D, dtype=np.float16)
    # S stored partition-contiguous: s_tab[e, g*128 + d] = (dst[g*128+e] == d)
    # so each device load is 128 large contiguous descriptors, not 128*cnt
    # small strided ones.
    s_tab = np.stack(
        [
            (dst_reb[c].reshape(gtot, 128)[:, :, None] == dgrid[None, None, :])
            .astype(np.float16).transpose(1, 0, 2).reshape(D, nslot)
            for c in range(C)
        ]
    )                                                  # (C, 128, nslot)
    st_tab = np.stack(
        [(dst_reb[c][None, :] == dgrid[:, None]).astype(np.float16)
         for c in range(C)]
    )                                                  # (C, 128, nslot)

    geom = {
        "G": G,
        "base": base,
        "gtot": gtot,
        "batches": batches,
        "binfo": binfo,
        "gt_max": int((G[:, 0] + G[:, 1]).max()),
        "gb_max": int(max(e - s for (s, _h, e) in binfo)),
    }
    return geom, src_w, s_tab, st_tab


def prep_weights(cfg, Ws, als, ars, bs):
    """Combined matmul weights Wc = [W | wl | wr] and packed bias rows.

    W[l] is (D, H*F) with head-major columns; wl[k,h] = sum_f W[k,h,f]*al[h,f]
    folds the attention dot products into the same matmul.
    """
    L = cfg.layers
    wc = np.zeros((L, D, HF + 4), dtype=np.float16)
    bp = np.zeros((L, 1, 384), dtype=np.float16)
    for l in range(L):
        W = np.asarray(Ws[l], dtype=np.float32)            # (D, H*F)
        Wh = W.reshape(D, H, D)                            # (D, H, F)
        wl = np.einsum("khf,hf->kh", Wh, np.asarray(als[l], np.float32))
        wr = np.einsum("khf,hf->kh", Wh, np.asarray(ars[l], np.float32))
        wc[l, :, :HF] = W.astype(np.float16)
        wc[l, :, HF : HF + 2] = wl.astype(np.float16)
        wc[l, :, HF + 2 : HF + 4] = wr.astype(np.float16)
        b = np.asarray(bs[l], np.float32)                  # (H, F)
        bp[l, 0, 0:128] = b[0].astype(np.float16)
        bp[l, 0, 128:256] = b[1].astype(np.float16)
        bp[l, 0, 256:384] = (0.5 * (b[0] + b[1])).astype(np.float16)
    return wc, bp


# ----------------------------------------------------------------------------
# Device kernel
# ----------------------------------------------------------------------------

def build(cfg, geom):
    C, T, L = N_CORES, cfg.t, cfg.layers
    G, base = geom["G"], geom["base"]
    gtot, batches, binfo = geom["gtot"], geom["batches"], geom["binfo"]
    GT_MAX, GB_MAX = geom["gt_max"], geom["gb_max"]
    nslot = gtot * 128
    NLO, NHI = cfg.split, cfg.n - cfg.split

    nc = bacc.Bacc("TRN2", target_bir_lowering=False, debug=False,
                   enable_asserts=False, num_devices=C, num_swdge_queues=2,
                   dynamic_dma_scratch_size=32768)

    # I/O
    xTb = nc.dram_tensor("xTb", [C, D, cfg.nloc], F16, kind="ExternalInput")
    wc_d = nc.dram_tensor("wc", [L, D, HF + 4], F16, kind="ExternalInput")
    bp_d = nc.dram_tensor("bp", [L, 1, 384], F16, kind="ExternalInput")
    src_d = nc.dram_tensor("srcw", [D, nslot // 16], I16, kind="ExternalInput")
    s_d = nc.dram_tensor("s_tab", [D, nslot], F16, kind="ExternalInput")
    st_d = nc.dram_tensor("st_tab", [D, nslot], F16, kind="ExternalInput")
    oh_d = nc.dram_tensor("onehot", [D, 8], F32, kind="ExternalInput")
    # raw layer-2 accumulators: [num_h0 | num_h1 | den_h0 | den_h1];
    # normalization + bias happen on the host
    out_d = nc.dram_tensor("out", [cfg.nloc_pad, 258], F32, kind="ExternalOutput")

    # internal DRAM
    tb_lo = nc.dram_tensor("tb_lo", [NLO, ROW16], F16)
    tb_hi = nc.dram_tensor("tb_hi", [NHI, ROW16], F16)
    hT_own = nc.dram_tensor("hT_own", [D, cfg.nloc], F16)
    hT_all = nc.dram_tensor("hT_all", [C, D, cfg.nloc], F16, addr_space="Shared")

    assert NLO == 4 * cfg.nloc, "lo half must be cores 0..3"

    with tile.TileContext(nc) as tc:
        with (
            tc.tile_pool(name="const", bufs=1) as cpool,
            tc.tile_pool(name="work", bufs=2) as pool,
            tc.tile_pool(name="ps_a", bufs=2, space="PSUM") as ppa,
            tc.tile_pool(name="ps_b", bufs=2, space="PSUM") as ppb,
            tc.tile_pool(name="ps_c", bufs=2, space="PSUM") as ppc,
        ):
            # ---- constants ----
            src_sb = cpool.tile([D, nslot // 16], I16, tag="src_sb")
            nc.sync.dma_start(out=src_sb[:], in_=src_d[:])
            oh_sb = cpool.tile([D, 8], F32, tag="oh_sb")
            nc.sync.dma_start(out=oh_sb[:], in_=oh_d[:])
            wc_sb = cpool.tile([D, L * (HF + 4)], F16, tag="wc_sb")
            bp_sb = cpool.tile([1, L * 384], F16, tag="bp_sb")
            for l in range(L):
                nc.sync.dma_start(
                    out=wc_sb[:, l * (HF + 4) : (l + 1) * (HF + 4)], in_=wc_d[l]
                )
                nc.sync.dma_start(
                    out=bp_sb[:, l * 384 : (l + 1) * 384], in_=bp_d[l]
                )

            ones_row = cpool.tile([1, D], F16, tag="ones_row")
            nc.vector.memset(ones_row[:], 1.0)
            ident = cpool.tile([D, D], F16, tag="ident")
            make_identity(nc, ident[:])

            er_stage = cpool.tile([D, T, 16], F32, tag="er_stage")
            er_own = cpool.tile([D, T, 2], F32, tag="er_own")
            er_own16 = cpool.tile([D, T, 2], F16, tag="er_own16")
            hT_stage = cpool.tile([D, cfg.nloc_pad], F16, tag="hT_stage")
            brep = cpool.tile([D, 384], F32, tag="brep")
            brep16 = cpool.tile([D, 128], F16, tag="brep16")

            for l in range(L):
                # ---- bias broadcast to all partitions (PE trick) ----
                bps = ppa.tile([D, 384], F32, tag="ps1")
                nc.tensor.matmul(
                    bps[:], lhsT=ones_row[:], rhs=bp_sb[:, l * 384 : (l + 1) * 384],
                    start=True, stop=True,
                )
                nc.vector.tensor_copy(brep[:], bps[:])
                if l == 0:
                    nc.vector.tensor_copy(brep16[:], bps[:, 256:384])
                nc.vector.memset(er_stage[:], 0.0)

                # ---- P1: feat/el table build (lo half = cores 0..3 first) ----
                wcl = wc_sb[:, l * (HF + 4) : l * (HF + 4) + HF + 4]
                scope = nc.named_scope(f"p1_l{l}")
                scope.__enter__()
                for cb in range(C):
                    tb = tb_lo if cb < 4 else tb_hi
                    nb0 = cb * cfg.nloc - (0 if cb < 4 else NLO)
                    for bt in range(0, T, P1B):
                        ntl = min(P1B, T - bt)
                        # partial last tile handled separately (store shape)
                        full = ntl if bt + ntl < T else ntl - 1
                        w_tot = full * 128 + (
                            0 if bt + ntl < T else cfg.w_last
                        )
                        xt = pool.tile([D, P1B * 128], F16, tag="xt")
                        if l == 0:
                            src_ap = xTb[cb, :, bt * 128 : bt * 128 + w_tot]
                        else:
                            src_ap = hT_all[cb, :, bt * 128 : bt * 128 + w_tot]
                        nc.sync.dma_start(out=xt[:, :w_tot], in_=src_ap)
                        stage = pool.tile([D, P1B, ROW16], F16, tag="stage")
                        for j in range(ntl):
                            w = 128 if bt + j < T - 1 else cfg.w_last
                            ps1 = ppa.tile([D, 384], F32, tag="ps1")
                            nc.tensor.matmul(
                                ps1[:w, 0 : HF + 4],
                                lhsT=xt[:, j * 128 : j * 128 + w],
                                rhs=wcl, start=True, stop=True,
                            )
                            # PSUM->SBUF copies alternate DVE/ACT (GPSIMD
                            # cannot read PSUM)
                            if j % 2 == 0:
                                nc.vector.tensor_copy(
                                    stage[:w, j, 0:HF], ps1[:w, 0:HF]
                                )
                                nc.vector.tensor_copy(
                                    stage[:w, j, HF : HF + 4].bitcast(F32),
                                    ps1[:w, HF : HF + 2],
                                )
                            else:
                                nc.scalar.activation(
                                    stage[:w, j, 0:HF], ps1[:w, 0:HF],
                                    mybir.ActivationFunctionType.Copy,
                                )
                                nc.scalar.activation(
                                    stage[:w, j, HF : HF + 4].bitcast(F32),
                                    ps1[:w, HF : HF + 2],
                                    mybir.ActivationFunctionType.Copy,
                                )
                            erv = er_stage[:, :, :].rearrange(
                                "p t (h k) -> p t h k", k=8
                            )
                            nc.vector.tensor_copy(
                                erv[:w, bt + j, :, cb], ps1[:w, HF + 2 : HF + 4]
                            )
                        # batched store of the full tiles, partial tile alone
                        if full > 0:
                            n0 = nb0 + bt * 128
                            nc.sync.dma_start(
                                out=tb[n0 : n0 + full * 128, :].rearrange(
                                    "(j p) e -> p j e", p=128
                                ),
                                in_=stage[:, 0:full, :],
                            )
                        if full < ntl:
                            n0 = nb0 + (bt + full) * 128
                            nc.sync.dma_start(
                                out=tb[n0 : n0 + cfg.w_last, :],
                                in_=stage[: cfg.w_last, full, :],
                            )

                # er_own = own core's column of er_stage
                er4 = er_stage[:, :, :].rearrange("p t (h k) -> p t h k", k=8)
                tmp_er = pool.tile([D, T, 2, 8], F32, tag="tmp_er")
                nc.vector.tensor_tensor(
                    out=tmp_er[:],
                    in0=er4,
                    in1=oh_sb[:].unsqueeze(1).unsqueeze(1).to_broadcast(
                        [D, T, 2, 8]
                    ),
                    op=mybir.AluOpType.mult,
                )
                nc.vector.reduce_sum(
                    er_own[:], tmp_er[:], axis=mybir.AxisListType.X
                )
                nc.vector.tensor_copy(er_own16[:], er_own[:])
                scope.__exit__(None, None, None)
                scope = nc.named_scope(f"p2_l{l}")
                scope.__enter__()

                # ---- P2: edge phase, batched gathers; the tile loop is
                # software-pipelined (finalize deferred two tiles) so finalize
                # ops that depend on the aggregation don't head-of-line block
                # the next tile's ready work in the in-order engine queues.
                def emit_pre(bi, t, gt):
                    g_lo0 = binfo[bi][0]
                    gl, gh = int(G[t, 0]), int(G[t, 1])
                    gt_n = gl + gh
                    rngs = [
                        (int(base[t, 0]) - g_lo0, gl, 0),
                        (int(base[t, 1]) - g_lo0, gh, gl),
                    ]
                    S = pool.tile([D, GT_MAX, D], F16, tag="S", bufs=3)
                    ST = pool.tile([D, GT_MAX, D], F16, tag="ST", bufs=3)
                    msg = pool.tile([D, GT_MAX, 258], F16, tag="msg", bufs=3)
                    ere = pool.tile([D, GT_MAX, 2], F32, tag="ere", bufs=3)
                    lr = pool.tile([D, GT_MAX, 2], F32, tag="lr", bufs=3)
                    erp = ppc.tile([D, GT_MAX, 2], F32, tag="erp", bufs=3)
                    for (bg0, cnt, k0) in rngs:
                        ga = g_lo0 + bg0  # absolute group index
                        nc.sync.dma_start(
                            out=S[:, k0 : k0 + cnt, :],
                            in_=s_d[:, ga * 128 : (ga + cnt) * 128]
                            .rearrange("p (g d) -> p g d", d=128),
                        )
                        nc.sync.dma_start(
                            out=ST[:, k0 : k0 + cnt, :],
                            in_=st_d[:, ga * 128 : (ga + cnt) * 128]
                            .rearrange("p (g e) -> p g e", e=128),
                        )
                        # er per edge: one small matmul per group
                        for k in range(cnt):
                            nc.tensor.matmul(
                                erp[:, k0 + k, :],
                                lhsT=ST[:, k0 + k, :],
                                rhs=er_own16[:, t, :],
                                start=True, stop=True,
                            )
                        elv = gt[:, bg0 : bg0 + cnt, HF : HF + 4].bitcast(F32)
                        nc.vector.tensor_tensor(
                            out=ere[:, k0 : k0 + cnt, :],
                            in0=erp[:, k0 : k0 + cnt, :],
                            in1=elv,
                            op=mybir.AluOpType.add,
                        )
                    # w = exp(lrelu(u)); exp on the Scalar engine, written
                    # directly into msg's denominator columns
                    nc.vector.tensor_scalar_mul(
                        lr[:, 0:gt_n, :], ere[:, 0:gt_n, :], NEG_SLOPE
                    )
                    nc.vector.tensor_tensor(
                        out=ere[:, 0:gt_n, :], in0=ere[:, 0:gt_n, :],
                        in1=lr[:, 0:gt_n, :], op=mybir.AluOpType.max,
                    )
                    nc.scalar.activation(
                        msg[:, 0:gt_n, 256:258], ere[:, 0:gt_n, :],
                        mybir.ActivationFunctionType.Exp,
                    )
                    for (bg0, cnt, k0) in rngs:
                        nc.vector.tensor_tensor(
                            out=msg[:, k0 : k0 + cnt, 0:256].rearrange(
                                "p g (h f) -> p g h f", h=2
                            ),
                            in0=gt[:, bg0 : bg0 + cnt, 0:256].rearrange(
                                "p g (h f) -> p g h f", h=2
                            ),
                            in1=msg[:, k0 : k0 + cnt, 256:258]
                            .unsqueeze(3).to_broadcast([D, cnt, 2, 128]),
                            op=mybir.AluOpType.mult,
                        )
                    ps2 = ppb.tile([D, 258], F32, tag="ps2", bufs=3)
                    for k in range(gt_n):
                        nc.tensor.matmul(
                            ps2[:],
                            lhsT=S[:, k, :],
                            rhs=msg[:, k, :],
                            start=(k == 0),
                            stop=(k == gt_n - 1),
                        )
                    return ps2

                def emit_fin(t, ps2):
                    osb = pool.tile([D, 258], F32, tag="osb")
                    nc.vector.tensor_copy(osb[:], ps2[:])
                    if l == 0:
                        # h = 0.5*(n0*r0 + n1*r1) + bmean, transposed;
                        # all math on the SBUF copy (PSUM reads are slow)
                        rsb = pool.tile([D, 2], F32, tag="rsb")
                        nc.vector.tensor_scalar_max(
                            rsb[:], osb[:, 256:258], 1e-30
                        )
                        nc.vector.reciprocal(rsb[:], rsb[:])
                        rh = pool.tile([D, 2], F32, tag="rh")
                        nc.vector.tensor_scalar_mul(rh[:], rsb[:], 0.5)
                        t_0 = pool.tile([D, D], F16, tag="t0")
                        nc.vector.tensor_scalar_mul(
                            t_0[:], osb[:, 0:128], rh[:, 0:1]
                        )
                        t_1 = pool.tile([D, D], F16, tag="t1")
                        nc.vector.tensor_scalar_mul(
                            t_1[:], osb[:, 128:256], rh[:, 1:2]
                        )
                        nc.vector.tensor_tensor(
                            out=t_0[:], in0=t_0[:], in1=t_1[:],
                            op=mybir.AluOpType.add,
                        )
                        ht16 = pool.tile([D, D], F16, tag="ht16")
                        nc.vector.tensor_tensor(
                            out=ht16[:], in0=t_0[:], in1=brep16[:],
                            op=mybir.AluOpType.add,
                        )
                        pst = ppa.tile([D, 384], F32, tag="ps1")
                        pst16 = pst.bitcast(F16)
                        nc.tensor.transpose(pst16[:, 0:128], ht16[:], ident[:])
                        nc.vector.tensor_copy(
                            hT_stage[:, t * 128 : (t + 1) * 128],
                            pst16[:, 0:128],
                        )
                    else:
                        # raw sums out; host normalizes and adds bias
                        nc.sync.dma_start(
                            out=out_d[t * 128 : (t + 1) * 128, :], in_=osb[:]
                        )

                pending = []
                cur_bi = -1
                gt = None
                for bi, (t0, ntl) in enumerate(batches):
                    for j in range(ntl):
                        t = t0 + j
                        if bi != cur_bi:
                            g_lo0, g_hi0, g_end = binfo[bi]
                            n_lo = g_hi0 - g_lo0
                            n_all = g_end - g_lo0
                            gt = pool.tile([D, GB_MAX, ROW16], F16, tag="gt")
                            nc.gpsimd.dma_gather(
                                out_ap=gt[:, 0:n_lo, :],
                                in_ap=tb_lo[:, :],
                                idxs_ap=src_sb[:, g_lo0 * 8 : g_hi0 * 8],
                                num_idxs=n_lo * 128,
                                num_idxs_reg=n_lo * 128,
                                elem_size=ROW16,
                                queue_num=0,
                                single_packet=False,
                            )
                            nc.gpsimd.dma_gather(
                                out_ap=gt[:, n_lo:n_all, :],
                                in_ap=tb_hi[:, :],
                                idxs_ap=src_sb[:, g_hi0 * 8 : g_end * 8],
                                num_idxs=(n_all - n_lo) * 128,
                                num_idxs_reg=(n_all - n_lo) * 128,
                                elem_size=ROW16,
                                queue_num=1,
                                single_packet=False,
                            )
                            cur_bi = bi
                        ps2 = emit_pre(bi, t, gt)
                        pending.append((t, ps2))
                        if len(pending) > 2:
                            emit_fin(*pending.pop(0))
                for p in pending:
                    emit_fin(*p)

                scope.__exit__(None, None, None)
                # ---- inter-layer allgather ----
                if l == 0:
                    nc.sync.dma_start(
                        out=hT_own[:], in_=hT_stage[:, 0 : cfg.nloc]
                    )
                    with nc.named_scope("cc"):
                        nc.gpsimd.collective_compute(
                            "AllGather",
                            mybir.AluOpType.bypass,
                            replica_groups=[list(range(C))],
                            ins=[hT_own[:]],
                            outs=[hT_all[:]],
                        )
    nc.compile()
    return nc


# ----------------------------------------------------------------------------
# Entry point
# ----------------------------------------------------------------------------

def run_gat(cfg, x, Ws, als, ars, bs, src, dst, trace=False):
    geom, src_w, s_tab, st_tab = prep_edges(cfg, src, dst)
    wc, bp = prep_weights(cfg, Ws, als, ars, bs)

    x = np.asarray(x, dtype=np.float32)
    xTb = np.ascontiguousarray(
        x.reshape(N_CORES, cfg.nloc, D).transpose(0, 2, 1)
    ).astype(np.float16)

    onehots = []
    for c in range(N_CORES):
        oh = np.zeros((D, 8), dtype=np.float32)
        oh[:, c] = 1.0
        onehots.append(oh)

    nc = build(cfg, geom)
    in_maps = []
    for c in range(N_CORES):
        in_maps.append({
            "xTb": xTb,
            "wc": wc,
            "bp": bp,
            "srcw": src_w[c],
            "s_tab": s_tab[c],
            "st_tab": st_tab[c],
            "onehot": onehots[c],
        })
    res = run_bass_kernel_spmd(nc, in_maps, list(range(N_CORES)), trace=trace)
    outs = [res.results[c]["out"][: cfg.nloc] for c in range(N_CORES)]
    raw = np.concatenate(outs, axis=0).astype(np.float64)   # (n, 258)
    num = raw[:, 0:HF].reshape(cfg.n, H, D)
    den = np.maximum(raw[:, HF : HF + 2], 1e-30)            # (n, H)
    out = num / den[:, :, None] + np.asarray(bs[-1], np.float64)[None]
    return out.astype(np.float32), res


def kernel(x, Ws, als, ars, bs, src, dst):
    out, _ = run_gat(FULL, x, Ws, als, ars, bs, src, dst, trace=False)
    return out.astype(np.float32)


# revision 60
# speedup vs baseline: 1.4578x; 1.0187x over previous
"""Trainium2 Bass kernel for 2-layer GAT (nn_GAT_43765716746408).

Self-contained: hardcodes the problem geometry (50000 nodes, 800000 edges,
D=128, H=2 heads, F=128, 2 layers) and distributes across 8 NeuronCores by
dst-node partition.

Strategy per layer (SPMD across 8 cores, identical program, per-core data):
  - Replicated feature matmul (P1): every core computes feat = h @ [W|wl|wr]
    for ALL nodes (wl/wr fold the attention vectors al/ar into the matmul so
    el = feat@al, er = feat@ar come out as 4 extra columns), writing a packed
    row table T[n] = [feat fp16 (512B) | el f32 (8B) | pad] (768B rows),
    split into two DRAM tables (lo/hi node halves) so edge gathers on the lo
    half can start while the hi half is still being written.  er for the
    core's own nodes is kept in SBUF (er_own).
  - Edge phase (P2): edges are bucketed by (dst-tile-of-128, src-half) on the
    host (padded with dummy edges to uniform bucket sizes shared by all
    cores), and buckets are grouped into batches of a few dst tiles so each
    dma_gather instruction covers thousands of edges (SWDGE descriptor
    generation on GpSimd has a large per-instruction cost).  Per dst tile:
    one-hot S[e, d] = (dst[e] == d) via iota + is_equal, transposed one-hot
    S_T[d, e] via a host-replicated dst table + partition iota, er per edge
    via a small matmul (S_T^T @ er_own), w = exp(lrelu(el + er)) on the
    Scalar engine written directly into msg columns 256:258, msg = w * feat
    on Vector, and out[d] = sum_e S[e,d] * msg[e] via one 128x258 matmul per
    128-edge group accumulated in PSUM.  Finalize divides by the accumulated
    denominators (msg cols 256:258 aggregate to per-dst sums of w) and adds
    bias.
  - Between layers: h = mean over heads, transposed on-chip to (feat, node)
    layout and AllGather'd so every core has the full h for layer 2's
    replicated matmul.
"""

import sys

sys.path.insert(0, "/opt/trn_rl_repo")

import numpy as np

import concourse.bass as bass
import concourse.tile as tile
from concourse import bacc, mybir
from concourse.bass_utils import run_bass_kernel_spmd
from concourse.masks import make_identity

F32 = mybir.dt.float32
F16 = mybir.dt.float16
I16 = mybir.dt.int16

N_CORES = 8
D = 128          # model dim
H = 2            # heads
HF = 256         # H * F
ROW16 = 384      # fp16 elements per table row (512B feat + 8B el + pad = 768B)
NEG_SLOPE = 0.2
B_TILES = 2      # dst tiles per gather batch
P1B = 4          # node tiles per P1 load/store batch


class Cfg:
    def __init__(self, n_nodes, n_edges, n_layers=2):
        assert n_nodes % N_CORES == 0
        self.n = n_nodes
        self.e = n_edges
        self.layers = n_layers
        self.nloc = n_nodes // N_CORES
        self.t = -(-self.nloc // 128)          # dst tiles per core
        self.nloc_pad = self.t * 128
        self.w_last = self.nloc - 128 * (self.t - 1)
        self.split = n_nodes // 2              # lo/hi table split (int16 range)
        assert self.split < 32768 and (n_nodes - self.split) < 32768
        assert self.nloc_pad < 32768


FULL = Cfg(50000, 800000)


# ----------------------------------------------------------------------------
# Host-side edge preprocessing
# ----------------------------------------------------------------------------

def prep_edges(cfg, src, dst):
    """Bucket edges per core by (dst_tile, src_half); pad to shared sizes.

    Slot order is batch-major: for each batch of B_TILES dst tiles, first all
    lo buckets of the batch's tiles, then all hi buckets, so one dma_gather
    per (batch, half) covers a contiguous slot range.
    """
    C, T = N_CORES, cfg.t
    counts = np.zeros((C, T, 2), dtype=np.int64)
    per_core = []
    core_of = dst // cfg.nloc
    for c in range(C):
        sel = core_of == c
        es, ed = src[sel].astype(np.int64), dst[sel].astype(np.int64)
        dloc = ed - c * cfg.nloc
        t = dloc // 128
        half = (es >= cfg.split).astype(np.int64)
        # sort by (tile, half, src) for gather locality
        order = np.lexsort((es, half, t))
        es, dloc, t, half = es[order], dloc[order], t[order], half[order]
        np.add.at(counts[c], (t, half), 1)
        per_core.append((es, dloc, t, half))

    gmax_th = counts.max(axis=0)                       # (T, 2)
    G = np.maximum(1, -(-gmax_th // 128))              # groups per (t, half)

    batches = [(t0, min(B_TILES, T - t0)) for t0 in range(0, T, B_TILES)]
    base = np.zeros((T, 2), dtype=np.int64)            # group offset per bucket
    acc = 0
    binfo = []                                         # (g_lo0, g_hi0, g_end)
    for (t0, nt) in batches:
        g_lo0 = acc
        for t in range(t0, t0 + nt):
            base[t, 0] = acc
            acc += G[t, 0]
        g_hi0 = acc
        for t in range(t0, t0 + nt):
            base[t, 1] = acc
            acc += G[t, 1]
        binfo.append((g_lo0, g_hi0, acc))
    gtot = acc
    nslot = gtot * 128

    src_idx = np.zeros((C, nslot), dtype=np.int16)
    dst_reb = np.full((C, nslot), -1.0, dtype=np.float16)
    for c in range(C):
        es, dloc, t, half = per_core[c]
        # edges are lexsorted by (t, half, src) so buckets are contiguous:
        # position within bucket = arange - bucket start
        bucket_id = t * 2 + half
        n = len(es)
        starts = np.searchsorted(bucket_id, np.arange(T * 2), side="left")
        pos_in_bucket = np.arange(n) - starts[bucket_id]
        slot = base[t, half] * 128 + pos_in_bucket
        src_idx[c, slot] = (es - np.where(half == 1, cfg.split, 0)).astype(np.int16)
        dst_reb[c, slot] = (dloc - t * 128).astype(np.float32)

    # wrapped int16 index layout: element s -> [s % 16, s // 16], replicated
    # to 128 partitions (the 8 gpsimd cores each read their 16-partition copy)
    def wrap16(a):
        w = a.reshape(-1, 16).T.copy()                 # (16, nslot/16)
        return np.tile(w, (8, 1))                      # (128, nslot/16)

    src_w = np.stack([wrap16(src_idx[c]) for c in range(C)])
    # host-precomputed one-hot matrices (layer-invariant, streamed from DRAM):
    # S[slot, d]  = (dst_reb[slot] == d)   rows=slot, for lhsT of aggregation
    # ST[d, slot] = (dst_reb[slot] == d)   rows=d, for lhsT of er broadcast
    dgrid = np.arange(D, dtype=np.float16)
    # S stored partition-contiguous: s_tab[e, g*128 + d] = (dst[g*128+e] == d)
    # so each device load is 128 large contiguous descriptors, not 128*cnt
    # small strided ones.
    s_tab = np.stack(
        [
            (dst_reb[c].reshape(gtot, 128)[:, :, None] == dgrid[None, None, :])
            .astype(np.float16).transpose(1, 0, 2).reshape(D, nslot)
            for c in range(C)
        ]
    )                                                  # (C, 128, nslot)
    st_tab = np.stack(
        [(dst_reb[c][None, :] == dgrid[:, None]).astype(np.float16)
         for c in range(C)]
    )                                                  # (C, 128, nslot)

    geom = {
        "G": G,
        "base": base,
        "gtot": gtot,
        "batches": batches,
        "binfo": binfo,
        "gt_max": int((G[:, 0] + G[:, 1]).max()),
        "gb_max": int(max(e - s for (s, _h, e) in binfo)),
    }
    return geom, src_w, s_tab, st_tab


def prep_weights(cfg, Ws, als, ars, bs):
    """Combined matmul weights Wc = [W | wl | wr] and packed bias rows.

    W[l] is (D, H*F) with head-major columns; wl[k,h] = sum_f W[k,h,f]*al[h,f]
    folds the attention dot products into the same matmul.
    """
    L = cfg.layers
    wc = np.zeros((L, D, HF + 4), dtype=np.float16)
    bp = np.zeros((L, 1, 384), dtype=np.float16)
    for l in range(L):
        W = np.asarray(Ws[l], dtype=np.float32)            # (D, H*F)
        Wh = W.reshape(D, H, D)                            # (D, H, F)
        wl = np.einsum("khf,hf->kh", Wh, np.asarray(als[l], np.float32))
        wr = np.einsum("khf,hf->kh", Wh, np.asarray(ars[l], np.float32))
        wc[l, :, :HF] = W.astype(np.float16)
        wc[l, :, HF : HF + 2] = wl.astype(np.float16)
        wc[l, :, HF + 2 : HF + 4] = wr.astype(np.float16)
        b = np.asarray(bs[l], np.float32)                  # (H, F)
        bp[l, 0, 0:128] = b[0].astype(np.float16)
        bp[l, 0, 128:256] = b[1].astype(np.float16)
        bp[l, 0, 256:384] = (0.5 * (b[0] + b[1])).astype(np.float16)
    return wc, bp


# ----------------------------------------------------------------------------
# Device kernel
# ----------------------------------------------------------------------------

def build(cfg, geom):
    C, T, L = N_CORES, cfg.t, cfg.layers
    G, base = geom["G"], geom["base"]
    gtot, batches, binfo = geom["gtot"], geom["batches"], geom["binfo"]
    GT_MAX, GB_MAX = geom["gt_max"], geom["gb_max"]
    nslot = gtot * 128
    NLO, NHI = cfg.split, cfg.n - cfg.split

    nc = bacc.Bacc("TRN2", target_bir_lowering=False, debug=False,
                   enable_asserts=False, num_devices=C, num_swdge_queues=2,
                   dynamic_dma_scratch_size=32768)

    # I/O
    xTb = nc.dram_tensor("xTb", [C, D, cfg.nloc], F16, kind="ExternalInput")
    wc_d = nc.dram_tensor("wc", [L, D, HF + 4], F16, kind="ExternalInput")
    bp_d = nc.dram_tensor("bp", [L, 1, 384], F16, kind="ExternalInput")
    src_d = nc.dram_tensor("srcw", [D, nslot // 16], I16, kind="ExternalInput")
    s_d = nc.dram_tensor("s_tab", [D, nslot], F16, kind="ExternalInput")
    st_d = nc.dram_tensor("st_tab", [D, nslot], F16, kind="ExternalInput")
    oh_d = nc.dram_tensor("onehot", [D, 8], F32, kind="ExternalInput")
    # raw layer-2 accumulators: [num_h0 | num_h1 | den_h0 | den_h1];
    # normalization + bias happen on the host
    out_d = nc.dram_tensor("out", [cfg.nloc_pad, 258], F32, kind="ExternalOutput")

    # internal DRAM
    tb_lo = nc.dram_tensor("tb_lo", [NLO, ROW16], F16)
    tb_hi = nc.dram_tensor("tb_hi", [NHI, ROW16], F16)
    hT_own = nc.dram_tensor("hT_own", [D, cfg.nloc], F16)
    hT_all = nc.dram_tensor("hT_all", [C, D, cfg.nloc], F16, addr_space="Shared")

    assert NLO == 4 * cfg.nloc, "lo half must be cores 0..3"

    with tile.TileContext(nc) as tc:
        with (
            tc.tile_pool(name="const", bufs=1) as cpool,
            tc.tile_pool(name="work", bufs=2) as pool,
            tc.tile_pool(name="ps_a", bufs=2, space="PSUM") as ppa,
            tc.tile_pool(name="ps_b", bufs=2, space="PSUM") as ppb,
            tc.tile_pool(name="ps_c", bufs=2, space="PSUM") as ppc,
        ):
            # ---- constants ----
            src_sb = cpool.tile([D, nslot // 16], I16, tag="src_sb")
            nc.sync.dma_start(out=src_sb[:], in_=src_d[:])
            oh_sb = cpool.tile([D, 8], F32, tag="oh_sb")
            nc.sync.dma_start(out=oh_sb[:], in_=oh_d[:])
            wc_sb = cpool.tile([D, L * (HF + 4)], F16, tag="wc_sb")
            bp_sb = cpool.tile([1, L * 384], F16, tag="bp_sb")
            for l in range(L):
                nc.sync.dma_start(
                    out=wc_sb[:, l * (HF + 4) : (l + 1) * (HF + 4)], in_=wc_d[l]
                )
                nc.sync.dma_start(
                    out=bp_sb[:, l * 384 : (l + 1) * 384], in_=bp_d[l]
                )

            ones_row = cpool.tile([1, D], F16, tag="ones_row")
            nc.vector.memset(ones_row[:], 1.0)
            ident = cpool.tile([D, D], F16, tag="ident")
            make_identity(nc, ident[:])

            er_stage = cpool.tile([D, T, 16], F32, tag="er_stage")
            er_own = cpool.tile([D, T, 2], F32, tag="er_own")
            er_own16 = cpool.tile([D, T, 2], F16, tag="er_own16")
            hT_stage = cpool.tile([D, cfg.nloc_pad], F16, tag="hT_stage")
            brep = cpool.tile([D, 384], F32, tag="brep")
            brep16 = cpool.tile([D, 128], F16, tag="brep16")

            for l in range(L):
                # ---- bias broadcast to all partitions (PE trick) ----
                bps = ppa.tile([D, 384], F32, tag="ps1")
                nc.tensor.matmul(
                    bps[:], lhsT=ones_row[:], rhs=bp_sb[:, l * 384 : (l + 1) * 384],
                    start=True, stop=True,
                )
                nc.vector.tensor_copy(brep[:], bps[:])
                if l == 0:
                    nc.vector.tensor_copy(brep16[:], bps[:, 256:384])
                nc.vector.memset(er_stage[:], 0.0)

                # ---- P1: feat/el table build (lo half = cores 0..3 first) ----
                wcl = wc_sb[:, l * (HF + 4) : l * (HF + 4) + HF + 4]
                scope = nc.named_scope(f"p1_l{l}")
                scope.__enter__()
                for cb in range(C):
                    tb = tb_lo if cb < 4 else tb_hi
                    nb0 = cb * cfg.nloc - (0 if cb < 4 else NLO)
                    for bt in range(0, T, P1B):
                        ntl = min(P1B, T - bt)
                        # partial last tile handled separately (store shape)
                        full = ntl if bt + ntl < T else ntl - 1
                        w_tot = full * 128 + (
                            0 if bt + ntl < T else cfg.w_last
                        )
                        xt = pool.tile([D, P1B * 128], F16, tag="xt")
                        if l == 0:
                            src_ap = xTb[cb, :, bt * 128 : bt * 128 + w_tot]
                        else:
                            src_ap = hT_all[cb, :, bt * 128 : bt * 128 + w_tot]
                        nc.sync.dma_start(out=xt[:, :w_tot], in_=src_ap)
                        stage = pool.tile([D, P1B, ROW16], F16, tag="stage")
                        for j in range(ntl):
                            w = 128 if bt + j < T - 1 else cfg.w_last
                            ps1 = ppa.tile([D, 384], F32, tag="ps1")
                            nc.tensor.matmul(
                                ps1[:w, 0 : HF + 4],
                                lhsT=xt[:, j * 128 : j * 128 + w],
                                rhs=wcl, start=True, stop=True,
                            )
                            # PSUM->SBUF copies alternate DVE/ACT (GPSIMD
                            # cannot read PSUM)
                            if j % 2 == 0:
                                nc.vector.tensor_copy(
                                    stage[:w, j, 0:HF], ps1[:w, 0:HF]
                                )
                                nc.vector.tensor_copy(
                                    stage[:w, j, HF : HF + 4].bitcast(F32),
                                    ps1[:w, HF : HF + 2],
                                )
                            else:
                                nc.scalar.activation(
                                    stage[:w, j, 0:HF], ps1[:w, 0:HF],
                                    mybir.ActivationFunctionType.Copy,
                                )
                                nc.scalar.activation(
                                    stage[:w, j, HF : HF + 4].bitcast(F32),
                                    ps1[:w, HF : HF + 2],
                                    mybir.ActivationFunctionType.Copy,
                                )
                            erv = er_stage[:, :, :].rearrange(
                                "p t (h k) -> p t h k", k=8
                            )
                            nc.vector.tensor_copy(
                                erv[:w, bt + j, :, cb], ps1[:w, HF + 2 : HF + 4]
                            )
                        # batched store of the full tiles, partial tile alone
                        if full > 0:
                            n0 = nb0 + bt * 128
                            nc.sync.dma_start(
                                out=tb[n0 : n0 + full * 128, :].rearrange(
                                    "(j p) e -> p j e", p=128
                                ),
                                in_=stage[:, 0:full, :],
                            )
                        if full < ntl:
                            n0 = nb0 + (bt + full) * 128
                            nc.sync.dma_start(
                                out=tb[n0 : n0 + cfg.w_last, :],
                                in_=stage[: cfg.w_last, full, :],
                            )

                # er_own = own core's column of er_stage
                er4 = er_stage[:, :, :].rearrange("p t (h k) -> p t h k", k=8)
                tmp_er = pool.tile([D, T, 2, 8], F32, tag="tmp_er")
                nc.vector.tensor_tensor(
                    out=tmp_er[:],
                    in0=er4,
                    in1=oh_sb[:].unsqueeze(1).unsqueeze(1).to_broadcast(
                        [D, T, 2, 8]
                    ),
                    op=mybir.AluOpType.mult,
                )
                nc.vector.reduce_sum(
                    er_own[:], tmp_er[:], axis=mybir.AxisListType.X
                )
                nc.vector.tensor_copy(er_own16[:], er_own[:])
                scope.__exit__(None, None, None)
                scope = nc.named_scope(f"p2_l{l}")
                scope.__enter__()

                # ---- P2: edge phase, batched gathers; the tile loop is
                # software-pipelined (finalize deferred two tiles) so finalize
                # ops that depend on the aggregation don't head-of-line block
                # the next tile's ready work in the in-order engine queues.
                def emit_pre(bi, t, gt):
                    g_lo0 = binfo[bi][0]
                    gl, gh = int(G[t, 0]), int(G[t, 1])
                    gt_n = gl + gh
                    rngs = [
                        (int(base[t, 0]) - g_lo0, gl, 0),
                        (int(base[t, 1]) - g_lo0, gh, gl),
                    ]
                    S = pool.tile([D, GT_MAX, D], F16, tag="S", bufs=3)
                    ST = pool.tile([D, GT_MAX, D], F16, tag="ST", bufs=3)
                    msg = pool.tile([D, GT_MAX, 258], F16, tag="msg", bufs=3)
                    ere = pool.tile([D, GT_MAX, 2], F32, tag="ere", bufs=3)
                    lr = pool.tile([D, GT_MAX, 2], F32, tag="lr", bufs=3)
                    erp = ppc.tile([D, GT_MAX, 2], F32, tag="erp", bufs=3)
                    for (bg0, cnt, k0) in rngs:
                        ga = g_lo0 + bg0  # absolute group index
                        nc.sync.dma_start(
                            out=S[:, k0 : k0 + cnt, :],
                            in_=s_d[:, ga * 128 : (ga + cnt) * 128]
                            .rearrange("p (g d) -> p g d", d=128),
                        )
                        nc.scalar.dma_start(
                            out=ST[:, k0 : k0 + cnt, :],
                            in_=st_d[:, ga * 128 : (ga + cnt) * 128]
                            .rearrange("p (g e) -> p g e", e=128),
                        )
                        # er per edge: one small matmul per group
                        for k in range(cnt):
                            nc.tensor.matmul(
                                erp[:, k0 + k, :],
                                lhsT=ST[:, k0 + k, :],
                                rhs=er_own16[:, t, :],
                                start=True, stop=True,
                            )
                        elv = gt[:, bg0 : bg0 + cnt, HF : HF + 4].bitcast(F32)
                        nc.vector.tensor_tensor(
                            out=ere[:, k0 : k0 + cnt, :],
                            in0=erp[:, k0 : k0 + cnt, :],
                            in1=elv,
                            op=mybir.AluOpType.add,
                        )
                    # w = exp(lrelu(u)); exp on the Scalar engine, written
                    # directly into msg's denominator columns
                    nc.vector.tensor_scalar_mul(
                        lr[:, 0:gt_n, :], ere[:, 0:gt_n, :], NEG_SLOPE
                    )
                    nc.vector.tensor_tensor(
                        out=ere[:, 0:gt_n, :], in0=ere[:, 0:gt_n, :],
                        in1=lr[:, 0:gt_n, :], op=mybir.AluOpType.max,
                    )
                    nc.scalar.activation(
                        msg[:, 0:gt_n, 256:258], ere[:, 0:gt_n, :],
                        mybir.ActivationFunctionType.Exp,
                    )
                    for (bg0, cnt, k0) in rngs:
                        nc.vector.tensor_tensor(
                            out=msg[:, k0 : k0 + cnt, 0:256].rearrange(
                                "p g (h f) -> p g h f", h=2
                            ),
                            in0=gt[:, bg0 : bg0 + cnt, 0:256].rearrange(
                                "p g (h f) -> p g h f", h=2
                            ),
                            in1=msg[:, k0 : k0 + cnt, 256:258]
                            .unsqueeze(3).to_broadcast([D, cnt, 2, 128]),
                            op=mybir.AluOpType.mult,
                        )
                    ps2 = ppb.tile([D, 258], F32, tag="ps2", bufs=3)
                    for k in range(gt_n):
                        nc.tensor.matmul(
                            ps2[:],
                            lhsT=S[:, k, :],
                            rhs=msg[:, k, :],
                            start=(k == 0),
                            stop=(k == gt_n - 1),
                        )
                    return ps2

                def emit_fin(t, ps2):
                    osb = pool.tile([D, 258], F32, tag="osb")
                    nc.vector.tensor_copy(osb[:], ps2[:])
                    if l == 0:
                        # h = 0.5*(n0*r0 + n1*r1) + bmean, transposed;
                        # all math on the SBUF copy (PSUM reads are slow)
                        rsb = pool.tile([D, 2], F32, tag="rsb")
                        nc.vector.tensor_scalar_max(
                            rsb[:], osb[:, 256:258], 1e-30
                        )
                        nc.vector.reciprocal(rsb[:], rsb[:])
                        rh = pool.tile([D, 2], F32, tag="rh")
                        nc.vector.tensor_scalar_mul(rh[:], rsb[:], 0.5)
                        t_0 = pool.tile([D, D], F16, tag="t0")
                        nc.vector.tensor_scalar_mul(
                            t_0[:], osb[:, 0:128], rh[:, 0:1]
                        )
                        t_1 = pool.tile([D, D], F16, tag="t1")
                        nc.vector.tensor_scalar_mul(
                            t_1[:], osb[:, 128:256], rh[:, 1:2]
                        )
                        nc.vector.tensor_tensor(
                            out=t_0[:], in0=t_0[:], in1=t_1[:],
                            op=mybir.AluOpType.add,
                        )
                        ht16 = pool.tile([D, D], F16, tag="ht16")
                        nc.vector.tensor_tensor(
                            out=ht16[:], in0=t_0[:], in1=brep16[:],
                            op=mybir.AluOpType.add,
                        )
                        pst = ppa.tile([D, 384], F32, tag="ps1")
                        pst16 = pst.bitcast(F16)
                        nc.tensor.transpose(pst16[:, 0:128], ht16[:], ident[:])
                        nc.vector.tensor_copy(
                            hT_stage[:, t * 128 : (t + 1) * 128],
                            pst16[:, 0:128],
                        )
                    else:
                        # raw sums out; host normalizes and adds bias
                        nc.sync.dma_start(
                            out=out_d[t * 128 : (t + 1) * 128, :], in_=osb[:]
                        )

                pending = []
                cur_bi = -1
                gt = None
                for bi, (t0, ntl) in enumerate(batches):
                    for j in range(ntl):
                        t = t0 + j
                        if bi != cur_bi:
                            g_lo0, g_hi0, g_end = binfo[bi]
                            n_lo = g_hi0 - g_lo0
                            n_all = g_end - g_lo0
                            gt = pool.tile([D, GB_MAX, ROW16], F16, tag="gt")
                            nc.gpsimd.dma_gather(
                                out_ap=gt[:, 0:n_lo, :],
                                in_ap=tb_lo[:, :],
                                idxs_ap=src_sb[:, g_lo0 * 8 : g_hi0 * 8],
                                num_idxs=n_lo * 128,
                                num_idxs_reg=n_lo * 128,
                                elem_size=ROW16,
                                queue_num=0,
                                single_packet=False,
                            )
                            nc.gpsimd.dma_gather(
                                out_ap=gt[:, n_lo:n_all, :],
                                in_ap=tb_hi[:, :],
                                idxs_ap=src_sb[:, g_hi0 * 8 : g_end * 8],
                                num_idxs=(n_all - n_lo) * 128,
                                num_idxs_reg=(n_all - n_lo) * 128,
                                elem_size=ROW16,
                                queue_num=1,
                                single_packet=False,
                            )
                            cur_bi = bi
                        ps2 = emit_pre(bi, t, gt)
                        pending.append((t, ps2))
                        if len(pending) > 2:
                            emit_fin(*pending.pop(0))
                for p in pending:
                    emit_fin(*p)

                scope.__exit__(None, None, None)
                # ---- inter-layer allgather ----
                if l == 0:
                    nc.sync.dma_start(
                        out=hT_own[:], in_=hT_stage[:, 0 : cfg.nloc]
                    )
                    with nc.named_scope("cc"):
                        nc.gpsimd.collective_compute(
                            "AllGather",
                            mybir.AluOpType.bypass,
                            replica_groups=[list(range(C))],
                            ins=[hT_own[:]],
                            outs=[hT_all[:]],
                        )
    nc.compile()
    return nc


# ----------------------------------------------------------------------------
# Entry point
# ----------------------------------------------------------------------------

def run_gat(cfg, x, Ws, als, ars, bs, src, dst, trace=False):
    geom, src_w, s_tab, st_tab = prep_edges(cfg, src, dst)
    wc, bp = prep_weights(cfg, Ws, als, ars, bs)

    x = np.asarray(x, dtype=np.float32)
    xTb = np.ascontiguousarray(
        x.reshape(N_CORES, cfg.nloc, D).transpose(0, 2, 1)
    ).astype(np.float16)

    onehots = []
    for c in range(N_CORES):
        oh = np.zeros((D, 8), dtype=np.float32)
        oh[:, c] = 1.0
        onehots.append(oh)

    nc = build(cfg, geom)
    in_maps = []
    for c in range(N_CORES):
        in_maps.append({
            "xTb": xTb,
            "wc": wc,
            "bp": bp,
            "srcw": src_w[c],
            "s_tab": s_tab[c],
            "st_tab": st_tab[c],
            "onehot": onehots[c],
        })
    res = run_bass_kernel_spmd(nc, in_maps, list(range(N_CORES)), trace=trace)
    outs = [res.results[c]["out"][: cfg.nloc] for c in range(N_CORES)]
    raw = np.concatenate(outs, axis=0).astype(np.float64)   # (n, 258)
    num = raw[:, 0:HF].reshape(cfg.n, H, D)
    den = np.maximum(raw[:, HF : HF + 2], 1e-30)            # (n, H)
    out = num / den[:, :, None] + np.asarray(bs[-1], np.float64)[None]
    return out.astype(np.float32), res


def kernel(x, Ws, als, ars, bs, src, dst):
    out, _ = run_gat(FULL, x, Ws, als, ars, bs, src, dst, trace=False)
    return out.astype(np.float32)
